# revision 1
# baseline (speedup 1.0000x reference)
"""Trainium2 Bass kernel for nn_CommonFeatureExtractor.

Data-parallel over 8 NeuronCores: batch dim (4096) sharded into 8 x 512,
weights replicated. Inside each core everything is computed in the
"transposed" layout [feature_on_partitions, batch_free] so that all matmul
contractions (which run over the partition axis on the PE) need no on-chip
transposes: the host feeds x already transposed and the weights are natural
[din, dout] = [K, M] layout, which is exactly what the PE's lhsT wants.

Pipeline per core (B=512 samples):
  A) 5 encoder MLPs (fp32 data, fp32r matmuls), fps.T stored bf16 [128,20,512]
  B) stats: pair products/squares (bf16) -> PE ones-matmul partition
     reductions -> d[10,B], ss[5,B]; softmax over selected pairs via
     ln/exp trick; per-pair weights broadcast to [128,B] via K=1 matmuls
  C) masked aggregation: G_i = sum_{pairs p containing i} (prod_p>0)*wq_p
     (+ mean-fallback), common.T = sum_i fps_i.T * G_i; wsum.T likewise with
     learned softmax gate weights
  D) enhance (sigmoid gate) + fuse matmuls -> fused.T [512, 512] -> host
     transposes back and concatenates.
"""

import numpy as np

import concourse.bass as bass
import concourse.mybir as mybir
import concourse.tile as tile
from concourse import bacc
from concourse.bass_utils import run_bass_kernel_spmd

F32 = mybir.dt.float32
F32R = mybir.dt.float32r
BF16 = mybir.dt.bfloat16
FP16 = mybir.dt.float16
ALU = mybir.AluOpType
AF = mybir.ActivationFunctionType

N_CORES = 8
B = 4096
BC = B // N_CORES  # 512 samples per core
H = 512
P = 128

AP_D, MA_D, MB_D, MC_D, PH_D = 2048, 167, 2048, 2048, 27
# encoders: (name, din, padded K tiles, hidden dh, M tiles = dh/128)
ENCS = [
    ("ap", AP_D, 16, 512),
    ("ma", MA_D, 2, 256),
    ("mb", MB_D, 16, 512),
    ("mc", MC_D, 16, 512),
    ("ph", PH_D, 1, 128),
]
XT_K = sum(e[2] for e in ENCS)  # 51 padded k-tiles of x
XT_OFF = np.cumsum([0] + [e[2] for e in ENCS])[:-1]  # [0,16,18,34,50]

_I = [0, 0, 0, 0, 1, 1, 1, 2, 2, 3]
_J = [1, 2, 3, 4, 2, 3, 4, 3, 4, 4]
PAIR_IDX = {(_I[p], _J[p]): p for p in range(10)}
# compute order: small encoders first so most pair-stats overlap phase A
ORDER = ["ma", "ph", "ap", "mb", "mc"]
ENC_BY_NAME = {e[0]: (i, e) for i, e in enumerate(ENCS)}
# pairs containing encoder i
PAIRS_OF = [[p for p in range(10) if _I[p] == i or _J[p] == i] for i in range(5)]

# midsection elementwise dtype
MID = FP16


DEBUG = False


def build_bass():
    nc = bacc.Bacc("TRN2", target_bir_lowering=False, debug=False)

    # ---------------- DRAM I/O ----------------
    xt = nc.dram_tensor("xt", [XT_K * P, BC], F32, kind="ExternalInput")
    w1 = {}
    w2 = {}
    b1 = {}
    b2 = {}
    for name, _, K, dh in ENCS:
        w1[name] = nc.dram_tensor(f"w1_{name}", [K * P, dh], F32, kind="ExternalInput")
        w2[name] = nc.dram_tensor(f"w2_{name}", [dh, H], F32, kind="ExternalInput")
        b1[name] = nc.dram_tensor(f"b1_{name}", [P, dh // P], F32, kind="ExternalInput")
        b2[name] = nc.dram_tensor(f"b2_{name}", [P, 4], F32, kind="ExternalInput")
    wg_w = nc.dram_tensor("wg_w", [5 * H, 5], FP16, kind="ExternalInput")
    wg_b = nc.dram_tensor("wg_b", [5, 1], F32, kind="ExternalInput")
    pcat = nc.dram_tensor("pcat", [5, 10], FP16, kind="ExternalInput")
    esel = nc.dram_tensor("esel", [10, 10 * P], FP16, kind="ExternalInput")
    enh_w = nc.dram_tensor("enh_w", [H, H], FP16, kind="ExternalInput")
    enh_b = nc.dram_tensor("enh_b", [P, 4], F32, kind="ExternalInput")
    fus_w = nc.dram_tensor("fus_w", [2 * H, H], FP16, kind="ExternalInput")
    fus_b = nc.dram_tensor("fus_b", [P, 4], F32, kind="ExternalInput")
    out = nc.dram_tensor("out", [H, BC], F32, kind="ExternalOutput")
    dbg = {}
    if DEBUG:
        for nm, shape in [("fps16", [P, 20, BC]), ("stats", [10, BC]),
                          ("ss", [5, BC]), ("wq", [10, BC]), ("fpw", [5, BC]),
                          ("commonT", [P, 4, BC]), ("wsumT", [P, 4, BC]),
                          ("wqrep", [P, 10, BC]), ("mfall", [P, BC])]:
            dt = F32
            dbg[nm] = nc.dram_tensor(f"dbg_{nm}", shape, dt, kind="ExternalOutput")

    with tile.TileContext(nc) as tc:
        kernel_body(
            tc, xt, w1, w2, b1, b2, wg_w, wg_b, pcat, esel, enh_w, enh_b, fus_w, fus_b,
            out, dbg,
        )
    nc.compile()
    return nc


def kernel_body(
    tc, xt, w1, w2, b1, b2, wg_w, wg_b, pcat, esel, enh_w, enh_b, fus_w, fus_b, out,
    dbg={},
):
    nc = tc.nc

    import contextlib

    ctx = contextlib.ExitStack()
    with ctx:
        # -------- pools --------
        persist = ctx.enter_context(tc.tile_pool(name="persist", bufs=1))
        smalls = ctx.enter_context(tc.tile_pool(name="smalls", bufs=1))
        statrows = ctx.enter_context(tc.tile_pool(name="statrows", bufs=1))
        wide_pool = ctx.enter_context(tc.tile_pool(name="widep", bufs=2))
        gs_pool = ctx.enter_context(tc.tile_pool(name="gsp", bufs=1))
        psum_mm = ctx.enter_context(tc.tile_pool(name="psum_mm", bufs=4, space="PSUM"))
        psum_st = ctx.enter_context(tc.tile_pool(name="psum_st", bufs=3, space="PSUM"))
        psum_bc = ctx.enter_context(tc.tile_pool(name="psum_bc", bufs=1, space="PSUM"))
        prod_pool = ctx.enter_context(tc.tile_pool(name="prod", bufs=2))
        t_pool = ctx.enter_context(tc.tile_pool(name="tpool", bufs=2))
        xt_pool = ctx.enter_context(tc.tile_pool(name="xtp", bufs=2))
        w_pool = ctx.enter_context(tc.tile_pool(name="wp", bufs=2))
        h_pool = ctx.enter_context(tc.tile_pool(name="hp", bufs=1))
        gate_pool = ctx.enter_context(tc.tile_pool(name="gatep", bufs=1))

        # -------- persistent tiles --------
        fps16 = persist.tile([P, 20, BC], MID)  # fps.T, ktile = enc*4 + ht
        fps32 = persist.tile([P, 20, BC], F32)  # exact fps.T for d/ss stats
        wqrep = persist.tile([P, 10, BC], MID)
        fpwrep = persist.tile([P, 5, BC], MID)
        mfallrep = persist.tile([P, BC], MID)
        common = persist.tile([P, 4, BC], MID)
        wsum = persist.tile([P, 4, BC], MID)
        enh_sb = persist.tile([P, 4, BC], MID)
        stats = persist.tile([10, BC], F32)  # pair dots d
        ss_t = persist.tile([5, BC], MID)  # squared norms
        l5 = persist.tile([5, BC], MID)
        ones_colf = persist.tile([P, 1], F32)
        ones_col16 = persist.tile([P, 1], MID)
        ones_row16 = persist.tile([1, P], MID)
        pcat_sb = persist.tile([5, 10], MID)
        esel_sb = persist.tile([10, 10 * P], MID)
        biases = {}
        for name, _, K, dh in ENCS:
            biases[name] = (
                persist.tile([P, dh // P], F32, name=f"b1sb_{name}"),
                persist.tile([P, 4], F32, name=f"b2sb_{name}"),
            )
        wgb_sb = persist.tile([5, 1], F32)
        enhb_sb = persist.tile([P, 4], F32)
        fusb_sb = persist.tile([P, 4], F32)

        nc.vector.memset(ones_colf, 1.0)
        nc.vector.memset(ones_col16, 1.0)
        nc.vector.memset(ones_row16, 1.0)
        nc.sync.dma_start(pcat_sb, pcat.ap())
        nc.sync.dma_start(esel_sb, esel.ap())
        for name, _, K, dh in ENCS:
            nc.sync.dma_start(biases[name][0], b1[name].ap())
            nc.sync.dma_start(biases[name][1], b2[name].ap())
        nc.sync.dma_start(wgb_sb, wg_b.ap())
        nc.sync.dma_start(enhb_sb, enh_b.ap())
        nc.sync.dma_start(fusb_sb, fus_b.ap())

        xt_view = xt.ap().rearrange("(ko p) n -> p ko n", p=P)

        # ================= Phase A: encoders (+ interleaved stats) =========
        def stat_row_to(dst, row, ps, nm):
            srow = statrows.tile([1, BC], dst.dtype, tag="statrow", name=f"srow_{nm}")
            nc.scalar.activation(srow, ps, AF.Copy)
            nc.sync.dma_start(dst[row : row + 1, :], srow)

        def emit_d_group(p, engine):
            ps = psum_st.tile([1, BC], F32, tag="stps", name=f"d_{p}")
            for ht in range(4):
                pr = prod_pool.tile([P, BC], F32, tag="prodf")
                engine.tensor_mul(
                    pr, fps32[:, _I[p] * 4 + ht, :], fps32[:, _J[p] * 4 + ht, :]
                )
                nc.tensor.matmul(ps, ones_colf, pr, start=(ht == 0), stop=(ht == 3))
            stat_row_to(stats, p, ps, f"d{p}")

        def emit_ss_group(i):
            ps = psum_st.tile([1, BC], F32, tag="stps", name=f"ss_{i}")
            for ht in range(4):
                sq = prod_pool.tile([P, BC], MID, tag="sq16")
                nc.scalar.square(sq, fps32[:, i * 4 + ht, :])
                nc.tensor.matmul(ps, ones_col16, sq, start=(ht == 0), stop=(ht == 3))
            stat_row_to(ss_t, i, ps, f"ss{i}")

        done_encs = []
        for name in ORDER:
            ei, (_, _, K, dh) = ENC_BY_NAME[name]
            M = dh // P
            b1_sb, b2_sb = biases[name]
            # ---- layer 1: h.T[dh, BC] = relu(w1.T @ x.T + b1) ----
            psums = [
                psum_mm.tile([P, BC], F32, tag="mmps", name=f"l1_{name}_{m}")
                for m in range(M)
            ]
            h_sb = h_pool.tile([P, 4, BC], F32, tag="htile")
            kdone = 0
            for kc0 in range(0, K, 4):
                kn = min(4, K - kc0)
                xt_t = xt_pool.tile([P, 4, BC], F32, tag="xt")
                nc.sync.dma_start(
                    xt_t[:, :kn, :],
                    xt_view[:, XT_OFF[ei] + kc0 : XT_OFF[ei] + kc0 + kn, :],
                )
                w1_t = w_pool.tile([P, 4, 512], F32, tag="w1")
                nc.sync.dma_start(
                    w1_t[:, :kn, :dh],
                    w1[name].ap()[kc0 * P : (kc0 + kn) * P, :].rearrange(
                        "(ko p) m -> p ko m", p=P
                    ),
                )
                for m in range(M):
                    for k in range(kn):
                        nc.tensor.matmul(
                            psums[m],
                            w1_t[:, k, m * P : (m + 1) * P],
                            xt_t[:, k, :],
                            start=(kdone + k == 0),
                            stop=(kdone + k == K - 1),
                        )
                kdone += kn
            for m in range(M):
                nc.scalar.activation(
                    h_sb[:, m, :], psums[m], AF.Relu, bias=b1_sb[:, m : m + 1]
                )
            # ---- layer 2: fps.T[H, BC] = w2.T @ h.T + b2 ----
            w2_t = w_pool.tile([P, 4, 512], F32, tag="w1")
            nc.sync.dma_start(
                w2_t[:, :M, :], w2[name].ap().rearrange("(ko p) m -> p ko m", p=P)
            )
            for m in range(4):
                ps = psum_mm.tile([P, BC], F32, tag="mmps", name=f"l2_{name}_{m}")
                for k in range(M):
                    nc.tensor.matmul(
                        ps,
                        w2_t[:, k, m * P : (m + 1) * P],
                        h_sb[:, k, :],
                        start=(k == 0),
                        stop=(k == M - 1),
                    )
                nc.scalar.activation(
                    fps32[:, ei * 4 + m, :], ps, AF.Identity, bias=b2_sb[:, m : m + 1]
                )
                nc.scalar.activation(
                    fps16[:, ei * 4 + m, :], ps, AF.Identity, bias=b2_sb[:, m : m + 1]
                )
            # ---- interleaved stats for this encoder + completed pairs ----
            emit_ss_group(ei)
            for prev in done_encs:
                pkey = (min(prev, ei), max(prev, ei))
                p = PAIR_IDX[pkey]
                # pairs completed before the last encoder overlap phase A on
                # GpSimd; the final encoder's pairs go to the (then-idle) DVE
                eng = nc.gpsimd if name != ORDER[-1] else nc.vector
                emit_d_group(p, eng)
            done_encs.append(ei)

        # ================= Phase B: softmax over selected pairs ============
        # ln of squared norms, then pairlog[p] = ln(ss_I) + ln(ss_J)
        nc.scalar.activation(l5, ss_t, AF.Ln)
        pl_ps = psum_st.tile([10, BC], F32, tag="stps", name="pl")
        nc.tensor.matmul(pl_ps, pcat_sb, l5, start=True, stop=True)
        invnn = smalls.tile([10, BC], MID)  # 1/(norm_I*norm_J)
        nc.scalar.activation(invnn, pl_ps, AF.Exp, scale=-0.5)
        sims = smalls.tile([10, BC], MID)
        nc.vector.tensor_mul(sims, stats[0:10, :], invnn)
        e0 = smalls.tile([10, BC], MID)
        nc.scalar.activation(e0, sims, AF.Exp)
        e_sb = smalls.tile([10, BC], MID)
        # e = (d > 0) * exp(sims)
        nc.vector.scalar_tensor_tensor(
            e_sb, in0=stats[0:10, :], scalar=0.0, in1=e0, op0=ALU.is_gt, op1=ALU.mult
        )
        den_ps = psum_st.tile([1, BC], F32, tag="stps", name="den")
        nc.tensor.matmul(den_ps, ones_col16[0:10, :], e_sb, start=True, stop=True)
        # mean-fallback weight row: 0.2 * (1 - any(sel))
        mfr = smalls.tile([1, BC], MID)
        nc.vector.tensor_scalar(
            mfr, in0=den_ps, scalar1=0.0, scalar2=-0.2, op0=ALU.is_gt, op1=ALU.mult
        )
        mfr2 = smalls.tile([1, BC], MID)
        nc.vector.tensor_scalar_add(mfr2, mfr, 0.2)
        mfr = mfr2
        # 1/denom on DVE (off the ACT critical path, no table switches);
        # denom is 0 (no sel) or > 1, so clamp at 1
        den_sb = smalls.tile([1, BC], F32)
        nc.vector.tensor_scalar_max(den_sb, den_ps, 1.0)
        recip = smalls.tile([1, BC], MID)
        with nc.allow_low_precision(reason="pair softmax weights tolerate fp16"):
            nc.vector.reciprocal(recip, den_sb)
        rr_ps = psum_st.tile([10, BC], F32, tag="stps", name="rr")
        nc.tensor.matmul(rr_ps, ones_row16[:, 0:10], recip, start=True, stop=True)
        wq_sb = smalls.tile([10, BC], MID)
        # wq = 0.5 * e / denom  (0.5 from the cf definition)
        nc.vector.scalar_tensor_tensor(
            wq_sb, in0=e_sb, scalar=0.5, in1=rr_ps, op0=ALU.mult, op1=ALU.mult
        )

        def broadcast(dst, src_tile, row, nm):
            # out[r, b] = sum_k esel[k, row*128+r] * src[k, b] = src[row, b]
            ksel = src_tile.shape[0]
            bc_ps = psum_bc.tile([P, BC], F32, tag="bcps", name=nm)
            nc.tensor.matmul(
                bc_ps,
                esel_sb[0:ksel, row * P : (row + 1) * P],
                src_tile,
                start=True,
                stop=True,
            )
            nc.scalar.activation(dst, bc_ps, AF.Copy)

        # learned per-fingerprint fusion weights fpw (softmax over 5)
        wg_sb = persist.tile([P, 20, 5], FP16)
        nc.sync.dma_start(wg_sb, wg_w.ap().rearrange("(ko p) m -> p ko m", p=P))
        z_ps = psum_st.tile([5, BC], F32, tag="stps", name="zgate")
        for kt in range(20):
            nc.tensor.matmul(
                z_ps, wg_sb[:, kt, :], fps16[:, kt, :], start=(kt == 0), stop=(kt == 19)
            )
        ez = smalls.tile([5, BC], MID)
        nc.scalar.activation(ez, z_ps, AF.Exp, bias=wgb_sb[0:5, :])
        sez_ps = psum_st.tile([1, BC], F32, tag="stps", name="sez")
        nc.tensor.matmul(sez_ps, ones_col16[0:5, :], ez, start=True, stop=True)
        rez = smalls.tile([1, BC], MID)
        sez_sb = smalls.tile([1, BC], F32, tag="lnrow", name="sez_sb")
        nc.scalar.activation(sez_sb, sez_ps, AF.Copy)
        with nc.allow_low_precision(reason="fusion softmax weights tolerate fp16"):
            nc.vector.reciprocal(rez, sez_sb)
        rz_ps = psum_st.tile([5, BC], F32, tag="stps", name="rz")
        nc.tensor.matmul(rz_ps, ones_row16[:, 0:5], rez, start=True, stop=True)
        fpw_sb = smalls.tile([5, BC], MID)
        nc.vector.tensor_mul(fpw_sb, ez, rz_ps)

        for i in range(5):
            broadcast(fpwrep[:, i, :], fpw_sb, i, f"bc_fpw{i}")
        for p in range(10):
            broadcast(wqrep[:, p, :], wq_sb, p, f"bc_wq{p}")
        broadcast(mfallrep, mfr, 0, "bc_mf")

        # ================= Phase C: masked aggregation =====================
        fps_by_ht = fps16.rearrange("p (i h) n -> p h i n", h=4)
        for ht in range(4):
            # pair products, all 10 in one wide tile
            prodw = wide_pool.tile([P, 10, BC], MID, tag="prodw")
            for p in range(10):
                nc.gpsimd.tensor_mul(
                    prodw[:, p, :],
                    fps16[:, _I[p] * 4 + ht, :],
                    fps16[:, _J[p] * 4 + ht, :],
                )
            # maskw_p = (prod_p > 0) * wq_p, one wide fused op
            maskw = wide_pool.tile([P, 10, BC], MID, tag="prodw", name=f"maskw{ht}")
            nc.vector.scalar_tensor_tensor(
                maskw, in0=prodw, scalar=0.0, in1=wqrep, op0=ALU.is_gt, op1=ALU.mult
            )
            # G_i = sum of the 4 maskw of pairs containing i, + mean-fallback
            # (pure tree, no in-place RMW: in-place DVE adds run ~3x slower)
            gs = gs_pool.tile([P, 5, BC], MID, tag="g")
            for i in range(5):
                pa, pb, pc_, pd = PAIRS_OF[i]
                ga = t_pool.tile([P, BC], MID, tag="gtmp", name=f"ga{ht}_{i}")
                gb = t_pool.tile([P, BC], MID, tag="gtmp2", name=f"gb{ht}_{i}")
                gc = t_pool.tile([P, BC], MID, tag="gtmp3", name=f"gc{ht}_{i}")
                nc.vector.tensor_add(ga, maskw[:, pa, :], maskw[:, pb, :])
                nc.vector.tensor_add(gb, maskw[:, pc_, :], maskw[:, pd, :])
                nc.vector.tensor_add(gc, ga, gb)
                nc.vector.tensor_add(gs[:, i, :], gc, mfallrep)
            # common.T[ht] = sum_i fps_i.T * G_i  (wide mult + pair tree)
            tuw = wide_pool.tile([P, 10, BC], MID, tag="prodw", name=f"tuw{ht}")
            tw = tuw[:, 0:5, :]
            uw = tuw[:, 5:10, :]
            nc.vector.tensor_mul(tw, fps_by_ht[:, ht, :, :], gs)
            r1 = t_pool.tile([P, 2, BC], MID, tag="r1", name=f"r1_{ht}")
            nc.vector.tensor_add(r1, tw[:, 0:4:2, :], tw[:, 1:4:2, :])
            r2 = t_pool.tile([P, BC], MID, tag="gtmp", name=f"r2_{ht}")
            nc.vector.tensor_add(r2, r1[:, 0, :], r1[:, 1, :])
            nc.vector.tensor_add(common[:, ht, :], r2, tw[:, 4, :])
            # wsum.T[ht] likewise with the learned fusion weights
            nc.vector.tensor_mul(uw, fps_by_ht[:, ht, :, :], fpwrep)
            u1 = t_pool.tile([P, 2, BC], MID, tag="r1", name=f"u1_{ht}")
            nc.vector.tensor_add(u1, uw[:, 0:4:2, :], uw[:, 1:4:2, :])
            u2 = t_pool.tile([P, BC], MID, tag="gtmp2", name=f"u2_{ht}")
            nc.vector.tensor_add(u2, u1[:, 0, :], u1[:, 1, :])
            nc.vector.tensor_add(wsum[:, ht, :], u2, uw[:, 4, :])

        if dbg:
            nc.gpsimd.dma_start(dbg["fps16"].ap(), fps16)
            nc.sync.dma_start(dbg["stats"].ap(), stats)
            nc.sync.dma_start(dbg["ss"].ap(), ss_t)
            nc.gpsimd.dma_start(dbg["wq"].ap(), wq_sb)
            nc.gpsimd.dma_start(dbg["fpw"].ap(), fpw_sb)
            nc.gpsimd.dma_start(dbg["commonT"].ap(), common)
            nc.gpsimd.dma_start(dbg["wsumT"].ap(), wsum)
            nc.gpsimd.dma_start(dbg["wqrep"].ap(), wqrep)
            nc.gpsimd.dma_start(dbg["mfall"].ap(), mfallrep)

        # ================= Phase D: enhance + fuse =================
        ew_t = w_pool.tile([P, 4, 512], FP16, tag="w16", name="ew_t")
        nc.sync.dma_start(ew_t, enh_w.ap().rearrange("(ko p) m -> p ko m", p=P))
        for m in range(4):
            ps = psum_mm.tile([P, BC], F32, tag="mmps", name=f"enh_{m}")
            for k in range(4):
                nc.tensor.matmul(
                    ps,
                    ew_t[:, k, m * P : (m + 1) * P],
                    common[:, k, :],
                    start=(k == 0),
                    stop=(k == 3),
                )
            gate = gate_pool.tile([P, BC], MID, tag="gate")
            nc.scalar.activation(gate, ps, AF.Sigmoid, bias=enhb_sb[:, m : m + 1])
            nc.vector.tensor_mul(enh_sb[:, m, :], common[:, m, :], gate)

        fw_view = fus_w.ap().rearrange("(ko p) m -> p ko m", p=P)
        fw_lo = w_pool.tile([P, 4, 512], FP16, tag="w16", name="fw_lo")
        nc.sync.dma_start(fw_lo, fw_view[:, 0:4, :])
        fw_hi = w_pool.tile([P, 4, 512], FP16, tag="w16", name="fw_hi")
        nc.sync.dma_start(fw_hi, fw_view[:, 4:8, :])
        out_view = out.ap().rearrange("(m p) n -> p m n", p=P)
        for m in range(4):
            ps = psum_mm.tile([P, BC], F32, tag="mmps", name=f"fus_{m}")
            for k in range(8):
                rhs = wsum[:, k, :] if k < 4 else enh_sb[:, k - 4, :]
                fw_t = fw_lo if k < 4 else fw_hi
                nc.tensor.matmul(
                    ps,
                    fw_t[:, k % 4, m * P : (m + 1) * P],
                    rhs,
                    start=(k == 0),
                    stop=(k == 7),
                )
            o_sb = gate_pool.tile([P, BC], F32, tag="osb")
            nc.scalar.activation(o_sb, ps, AF.Identity, bias=fusb_sb[:, m : m + 1])
            nc.sync.dma_start(out_view[:, m, :], o_sb)


def prep_inputs(inputs):
    """Host-side: build the per-core in_maps from full inputs."""
    x = np.asarray(inputs["fp_features"], np.float32)

    def pad_rows(a, rows):
        a = np.asarray(a, np.float32)
        if a.shape[0] == rows:
            return a
        out = np.zeros((rows, a.shape[1]), np.float32)
        out[: a.shape[0]] = a
        return out

    # padded transposed x, shared prep then per-core column slices
    xt_full = np.zeros((XT_K * P, B), np.float32)
    offs_in = np.cumsum([0, AP_D, MA_D, MB_D, MC_D])
    for ei, (name, din, K, dh) in enumerate(ENCS):
        seg = x[:, offs_in[ei] : offs_in[ei] + din]  # [B, din]
        xt_full[XT_OFF[ei] * P : XT_OFF[ei] * P + din, :] = np.ascontiguousarray(seg.T)

    common_map = {}
    for ei, (name, din, K, dh) in enumerate(ENCS):
        common_map[f"w1_{name}"] = pad_rows(inputs[f"{name}_w1"], K * P)
        common_map[f"w2_{name}"] = np.asarray(inputs[f"{name}_w2"], np.float32)
        common_map[f"b1_{name}"] = (
            np.asarray(inputs[f"{name}_b1"], np.float32).reshape(dh // P, P).T.copy()
        )
        common_map[f"b2_{name}"] = (
            np.asarray(inputs[f"{name}_b2"], np.float32).reshape(4, P).T.copy()
        )
    common_map["wg_w"] = np.asarray(inputs["wg_w"], np.float32).astype(np.float16)
    common_map["wg_b"] = np.asarray(inputs["wg_b"], np.float32).reshape(5, 1)
    pcat = np.zeros((5, 10), np.float32)
    for p in range(10):
        pcat[_I[p], p] = 1.0
        pcat[_J[p], p] = 1.0
    common_map["pcat"] = pcat.astype(np.float16)
    esel = np.zeros((10, 10 * 128), np.float16)
    for p in range(10):
        esel[p, p * 128 : (p + 1) * 128] = 1.0
    common_map["esel"] = esel
    common_map["enh_w"] = np.asarray(inputs["enh_w"], np.float16)
    common_map["enh_b"] = np.asarray(inputs["enh_b"], np.float32).reshape(4, P).T.copy()
    common_map["fus_w"] = np.asarray(inputs["fus_w"], np.float16)
    common_map["fus_b"] = np.asarray(inputs["fus_b"], np.float32).reshape(4, P).T.copy()

    in_maps = []
    for c in range(N_CORES):
        m = dict(common_map)
        m["xt"] = np.ascontiguousarray(xt_full[:, c * BC : (c + 1) * BC])
        in_maps.append(m)
    return in_maps


_NC_CACHE = None


def kernel(**inputs) -> np.ndarray:
    global _NC_CACHE
    if _NC_CACHE is None:
        _NC_CACHE = build_bass()
    nc = _NC_CACHE
    in_maps = prep_inputs(inputs)
    res = run_bass_kernel_spmd(nc, in_maps, core_ids=list(range(N_CORES)))
    outs = [res.results[c]["out"] for c in range(N_CORES)]  # each [H, BC]
    full = np.concatenate([o.T for o in outs], axis=0)  # [B, H]
    return np.ascontiguousarray(full.astype(np.float32))





# revision 10
# speedup vs baseline: 2.2590x; 2.2590x over previous
"""Trainium2 Bass kernel for nn_CommonFeatureExtractor (v2).

Data-parallel over 8 NeuronCores: batch dim (4096) sharded into 8 x 512,
weights replicated. Everything is computed in the transposed layout
[feature_on_partitions, batch_free].

v2 changes vs baseline:
  * all matmuls run in fp16 (fp32 LOW_HIGH pairs cost 4x on the PE)
  * pair products (fp16) are computed once on DVE/Pool during phase A and
    reused for both the d-statistics (ones-matmul partition reductions) and
    the masked pair features mcf_p = (prod_p>0)*(fps_I+fps_J), which are
    fully precomputed while the PE is busy with the encoder matmuls
  * softmax divisions are computed as exp(x - ln(denom)) where the
    subtraction is accumulated on the PE into PSUM (identity / -ones
    matmuls), avoiding the slow DVE reciprocal
  * phase C is a short chain of wide fp16 DVE ops: common = sum_p mcf_p *
    wqrep_p + fallback, interleaved with the enhance matmuls
"""

import numpy as np

import concourse.bass as bass
import concourse.mybir as mybir
import concourse.tile as tile
from concourse import bacc
from concourse.bass_utils import run_bass_kernel_spmd

F32 = mybir.dt.float32
FP16 = mybir.dt.float16
ALU = mybir.AluOpType
AF = mybir.ActivationFunctionType

N_CORES = 8
B = 4096
BC = B // N_CORES  # 512 samples per core
H = 512
P = 128

AP_D, MA_D, MB_D, MC_D, PH_D = 2048, 167, 2048, 2048, 27
# encoders: (name, din, padded K tiles, hidden dh)
ENCS = [
    ("ap", AP_D, 16, 512),
    ("ma", MA_D, 2, 256),
    ("mb", MB_D, 16, 512),
    ("mc", MC_D, 16, 512),
    ("ph", PH_D, 1, 128),
]
XT_K = sum(e[2] for e in ENCS)  # 51 padded k-tiles of x
XT_OFF = np.cumsum([0] + [e[2] for e in ENCS])[:-1]  # [0,16,18,34,50]

_I = [0, 0, 0, 0, 1, 1, 1, 2, 2, 3]
_J = [1, 2, 3, 4, 2, 3, 4, 3, 4, 4]
PAIR_IDX = {(_I[p], _J[p]): p for p in range(10)}
# compute order: small encoders first so most pair work overlaps phase A
ORDER = ["ma", "ph", "ap", "mb", "mc"]
ENC_BY_NAME = {e[0]: (i, e) for i, e in enumerate(ENCS)}
LAST_EI = ENC_BY_NAME[ORDER[-1]][0]

MID = FP16
LN_HALF = float(np.log(0.5))


def build_bass():
    nc = bacc.Bacc("TRN2", target_bir_lowering=False, debug=False)

    # ---------------- DRAM I/O ----------------
    xt = nc.dram_tensor("xt", [XT_K * P, BC], FP16, kind="ExternalInput")
    w1 = {}
    w2 = {}
    b1 = {}
    b2 = {}
    for name, _, K, dh in ENCS:
        w1[name] = nc.dram_tensor(f"w1_{name}", [K * P, dh], FP16, kind="ExternalInput")
        w2[name] = nc.dram_tensor(f"w2_{name}", [dh, H], FP16, kind="ExternalInput")
        b1[name] = nc.dram_tensor(f"b1_{name}", [P, dh // P], F32, kind="ExternalInput")
        b2[name] = nc.dram_tensor(f"b2_{name}", [P, 4], F32, kind="ExternalInput")
    wg_w = nc.dram_tensor("wg_w", [5 * H, 5], FP16, kind="ExternalInput")
    wg_b = nc.dram_tensor("wg_b", [5, 1], F32, kind="ExternalInput")
    pcat = nc.dram_tensor("pcat", [5, 10], FP16, kind="ExternalInput")
    esel = nc.dram_tensor("esel", [10, 10 * P], FP16, kind="ExternalInput")
    eye10 = nc.dram_tensor("eye10", [10, 10], FP16, kind="ExternalInput")
    mones10 = nc.dram_tensor("mones10", [1, 10], FP16, kind="ExternalInput")
    enh_w = nc.dram_tensor("enh_w", [H, H], FP16, kind="ExternalInput")
    enh_b = nc.dram_tensor("enh_b", [P, 4], F32, kind="ExternalInput")
    fus_w = nc.dram_tensor("fus_w", [2 * H, H], FP16, kind="ExternalInput")
    fus_b = nc.dram_tensor("fus_b", [P, 4], F32, kind="ExternalInput")
    out = nc.dram_tensor("out", [H, BC], F32, kind="ExternalOutput")

    with tile.TileContext(nc) as tc:
        kernel_body(
            tc, xt, w1, w2, b1, b2, wg_w, wg_b, pcat, esel, eye10, mones10,
            enh_w, enh_b, fus_w, fus_b, out,
        )
    nc.compile()
    return nc


def kernel_body(
    tc, xt, w1, w2, b1, b2, wg_w, wg_b, pcat, esel, eye10, mones10,
    enh_w, enh_b, fus_w, fus_b, out,
):
    nc = tc.nc

    import contextlib

    ctx = contextlib.ExitStack()
    with ctx:
        # -------- pools --------
        persist = ctx.enter_context(tc.tile_pool(name="persist", bufs=1))
        smalls = ctx.enter_context(tc.tile_pool(name="smalls", bufs=1))
        statrows = ctx.enter_context(tc.tile_pool(name="statrows", bufs=2))
        psum_mm = ctx.enter_context(tc.tile_pool(name="psum_mm", bufs=4, space="PSUM"))
        psum_st = ctx.enter_context(tc.tile_pool(name="psum_st", bufs=2, space="PSUM"))
        psum_z = ctx.enter_context(tc.tile_pool(name="psum_z", bufs=1, space="PSUM"))
        psum_bc = ctx.enter_context(tc.tile_pool(name="psum_bc", bufs=1, space="PSUM"))
        pair_pool = ctx.enter_context(tc.tile_pool(name="pairp", bufs=2))
        late_pool = ctx.enter_context(tc.tile_pool(name="latep", bufs=1))
        xt_pool = ctx.enter_context(tc.tile_pool(name="xtp", bufs=2))
        w_pool = ctx.enter_context(tc.tile_pool(name="wp", bufs=2))
        h_pool = ctx.enter_context(tc.tile_pool(name="hp", bufs=2))
        sq_pool = ctx.enter_context(tc.tile_pool(name="sqp", bufs=1))
        gate_pool = ctx.enter_context(tc.tile_pool(name="gatep", bufs=2))

        # -------- persistent tiles --------
        fps16 = persist.tile([P, 20, BC], MID)  # fps.T, ktile = enc*4 + ht
        mcfw = persist.tile([P, 10, 4, BC], MID)  # masked pair features
        wqrep = persist.tile([P, 10, BC], MID)
        fpwrep = persist.tile([P, 5, BC], MID)
        mfallrep = persist.tile([P, BC], MID)
        mpart = persist.tile([P, 4, BC], MID)  # partial sum of first-4 fps
        meansum = persist.tile([P, 4, BC], MID)  # sum of all 5 fps
        common = persist.tile([P, 4, BC], MID)
        wsum = persist.tile([P, 4, BC], MID)
        enh_sb = persist.tile([P, 4, BC], MID)
        stats = persist.tile([10, BC], MID)  # pair dots d
        ss_sb = persist.tile([5, BC], MID)  # squared norms
        ones_col16 = persist.tile([P, 1], MID)
        ln05 = persist.tile([10, 1], F32)
        pcat_sb = persist.tile([5, 10], MID)
        esel_sb = persist.tile([10, 10 * P], MID)
        eye10_sb = persist.tile([10, 10], MID)
        mones10_sb = persist.tile([1, 10], MID)
        biases = {}
        for name, _, K, dh in ENCS:
            biases[name] = (
                persist.tile([P, dh // P], F32, name=f"b1sb_{name}"),
                persist.tile([P, 4], F32, name=f"b2sb_{name}"),
            )
        wgb_sb = persist.tile([5, 1], F32)
        enhb_sb = persist.tile([P, 4], F32)
        fusb_sb = persist.tile([P, 4], F32)
        wg_sb = persist.tile([P, 20, 5], FP16)

        nc.vector.memset(ones_col16, 1.0)
        nc.vector.memset(ln05, LN_HALF)
        nc.sync.dma_start(pcat_sb, pcat.ap())
        nc.sync.dma_start(esel_sb, esel.ap())
        nc.sync.dma_start(eye10_sb, eye10.ap())
        nc.sync.dma_start(mones10_sb, mones10.ap())
        for name, _, K, dh in ENCS:
            nc.sync.dma_start(biases[name][0], b1[name].ap())
            nc.sync.dma_start(biases[name][1], b2[name].ap())
        nc.sync.dma_start(wgb_sb, wg_b.ap())
        nc.sync.dma_start(enhb_sb, enh_b.ap())
        nc.sync.dma_start(fusb_sb, fus_b.ap())
        nc.sync.dma_start(wg_sb, wg_w.ap().rearrange("(ko p) m -> p ko m", p=P))

        xt_view = xt.ap().rearrange("(ko p) n -> p ko n", p=P)

        # z-gate psum, accumulated across all 20 fps tiles during phase A
        z_ps = psum_z.tile([5, BC], F32, tag="zps", name="zgate")
        z_idx = [0]

        def stat_row_to(dst, row, ps, nm):
            srow = statrows.tile([1, BC], MID, tag="statrow", name=f"srow_{nm}")
            nc.scalar.activation(srow, ps, AF.Copy)
            nc.sync.dma_start(dst[row : row + 1, :], srow)

        def emit_d_group(p, prod_wide, nm):
            # d_p = sum_h prod over all 4 ht tiles (ones-matmul reduction)
            ps = psum_st.tile([1, BC], F32, tag="stps", name=f"d_{nm}")
            for ht in range(4):
                nc.tensor.matmul(
                    ps, ones_col16, prod_wide[:, ht, :], start=(ht == 0), stop=(ht == 3)
                )
            stat_row_to(stats, p, ps, f"d{p}")

        def emit_pair(pi, pj, engine, tagsuf):
            # wide (all-4-ht) ops: prod, s, mcf = (prod>0)*s; prod reused for d
            # (scalar_tensor_tensor only exists on DVE, so the mask-multiply
            # always runs there; prod/s can go to Pool)
            p = PAIR_IDX[(min(pi, pj), max(pi, pj))]
            fi = fps16[:, pi * 4 : pi * 4 + 4, :]
            fj = fps16[:, pj * 4 : pj * 4 + 4, :]
            prod = pair_pool.tile([P, 4, BC], MID, tag="prod", name=f"prod_{p}")
            engine.tensor_mul(prod, fi, fj)
            s_t = pair_pool.tile([P, 4, BC], MID, tag="s", name=f"s_{p}")
            engine.tensor_add(s_t, fi, fj)
            nc.vector.scalar_tensor_tensor(
                mcfw[:, p, :, :], in0=prod, scalar=0.0, in1=s_t,
                op0=ALU.is_gt, op1=ALU.mult,
            )
            emit_d_group(p, prod, f"p{p}")

        def emit_ss(ei):
            sq = sq_pool.tile([P, 4, BC], MID, tag="sq")
            nc.scalar.square(sq, fps16[:, ei * 4 : ei * 4 + 4, :])
            ps = psum_st.tile([1, BC], F32, tag="stps", name=f"ss_{ei}")
            for ht in range(4):
                nc.tensor.matmul(
                    ps, ones_col16, sq[:, ht, :], start=(ht == 0), stop=(ht == 3)
                )
            stat_row_to(ss_sb, ei, ps, f"ss{ei}")

        # ================= Phase A: encoders + interleaved pair work =======
        done_encs = []
        pair_count = [0]
        for name in ORDER:
            ei, (_, _, K, dh) = ENC_BY_NAME[name]
            M = dh // P
            b1_sb, b2_sb = biases[name]
            # ---- layer 1: h.T[dh, BC] = relu(w1.T @ x.T + b1) ----
            psums = [
                psum_mm.tile([P, BC], F32, tag="mmps", name=f"l1_{name}_{m}")
                for m in range(M)
            ]
            h_sb = h_pool.tile([P, 4, BC], MID, tag="htile")
            kdone = 0
            for kc0 in range(0, K, 4):
                kn = min(4, K - kc0)
                xt_t = xt_pool.tile([P, 4, BC], FP16, tag="xt")
                nc.sync.dma_start(
                    xt_t[:, :kn, :],
                    xt_view[:, XT_OFF[ei] + kc0 : XT_OFF[ei] + kc0 + kn, :],
                )
                w1_t = w_pool.tile([P, 4, 512], FP16, tag="w1")
                nc.sync.dma_start(
                    w1_t[:, :kn, :dh],
                    w1[name].ap()[kc0 * P : (kc0 + kn) * P, :].rearrange(
                        "(ko p) m -> p ko m", p=P
                    ),
                )
                for m in range(M):
                    for k in range(kn):
                        nc.tensor.matmul(
                            psums[m],
                            w1_t[:, k, m * P : (m + 1) * P],
                            xt_t[:, k, :],
                            start=(kdone + k == 0),
                            stop=(kdone + k == K - 1),
                        )
                kdone += kn
            for m in range(M):
                nc.scalar.activation(
                    h_sb[:, m, :], psums[m], AF.Relu, bias=b1_sb[:, m : m + 1]
                )
            # ---- layer 2: fps.T[H, BC] = w2.T @ h.T + b2 ----
            w2_t = w_pool.tile([P, 4, 512], FP16, tag="w1", name=f"w2_{name}")
            nc.sync.dma_start(
                w2_t[:, :M, :], w2[name].ap().rearrange("(ko p) m -> p ko m", p=P)
            )
            for m in range(4):
                ps = psum_mm.tile([P, BC], F32, tag="mmps", name=f"l2_{name}_{m}")
                for k in range(M):
                    nc.tensor.matmul(
                        ps,
                        w2_t[:, k, m * P : (m + 1) * P],
                        h_sb[:, k, :],
                        start=(k == 0),
                        stop=(k == M - 1),
                    )
                nc.scalar.activation(
                    fps16[:, ei * 4 + m, :], ps, AF.Identity, bias=b2_sb[:, m : m + 1]
                )
                # z-gate accumulation for the learned fusion weights
                nc.tensor.matmul(
                    z_ps,
                    wg_sb[:, ei * 4 + m, :],
                    fps16[:, ei * 4 + m, :],
                    start=(z_idx[0] == 0),
                    stop=(z_idx[0] == 19),
                )
                z_idx[0] += 1
            # ---- interleaved stats ----
            emit_ss(ei)
            if name != ORDER[-1]:
                for prev in done_encs:
                    # alternate DVE / Pool so both engines chew on pair work
                    eng = nc.gpsimd if pair_count[0] % 3 == 2 else nc.vector
                    emit_pair(prev, ei, eng, "g" if eng is nc.gpsimd else "v")
                    pair_count[0] += 1
                # partial mean-fallback sum (first 4 encoders)
                if len(done_encs) == 1:
                    nc.vector.tensor_add(
                        mpart,
                        fps16[:, done_encs[0] * 4 : done_encs[0] * 4 + 4, :],
                        fps16[:, ei * 4 : ei * 4 + 4, :],
                    )
                elif len(done_encs) >= 2:
                    nc.vector.tensor_add(
                        mpart, mpart, fps16[:, ei * 4 : ei * 4 + 4, :]
                    )
            done_encs.append(ei)

        # tail: pairs involving the last encoder (2 on DVE, 2 on Pool)
        last = LAST_EI
        tail_pairs = [PAIR_IDX[(min(prev, last), max(prev, last))] for prev in done_encs[:-1]]
        for idx, prev in enumerate(done_encs[:-1]):
            eng = nc.gpsimd if idx % 2 == 1 else nc.vector
            emit_pair(prev, last, eng, "g" if eng is nc.gpsimd else "v")
        # total mean-fallback sum
        nc.vector.tensor_add(meansum, mpart, fps16[:, last * 4 : last * 4 + 4, :])

        # ================= Phase B: pair softmax + fusion gate =============
        # l5 = ln(ss); pl = pcat @ l5; invnn = exp(-0.5*pl)
        l5 = smalls.tile([5, BC], MID)
        nc.scalar.activation(l5, ss_sb, AF.Ln)
        pl_ps = psum_st.tile([10, BC], F32, tag="stps", name="pl")
        nc.tensor.matmul(pl_ps, pcat_sb, l5, start=True, stop=True)
        invnn = smalls.tile([10, BC], MID)
        nc.scalar.activation(invnn, pl_ps, AF.Exp, scale=-0.5)
        sims = smalls.tile([10, BC], MID)
        nc.vector.tensor_mul(sims, stats[0:10, :], invnn)
        mask10 = smalls.tile([10, BC], MID)
        nc.vector.tensor_scalar(
            mask10, in0=stats[0:10, :], scalar1=0.0, scalar2=None, op0=ALU.is_gt
        )
        e0 = smalls.tile([10, BC], MID)
        nc.scalar.activation(e0, sims, AF.Exp)
        e_sb = smalls.tile([10, BC], MID)
        nc.vector.tensor_mul(e_sb, mask10, e0)
        den_ps = psum_st.tile([1, BC], F32, tag="stps", name="den")
        nc.tensor.matmul(den_ps, ones_col16[0:10, :], e_sb, start=True, stop=True)
        # mean-fallback row: 0.2 iff no pair selected
        mfr = smalls.tile([1, BC], MID)
        nc.vector.tensor_scalar(
            mfr, in0=den_ps, scalar1=0.0, scalar2=0.2, op0=ALU.is_le, op1=ALU.mult
        )
        # wq = 0.5 * mask * exp(sims - ln(max(den,1)))
        den_c = smalls.tile([1, BC], F32)
        nc.vector.tensor_scalar_max(den_c, den_ps, 1.0)
        lnden = smalls.tile([1, BC], MID)
        nc.scalar.activation(lnden, den_c, AF.Ln)
        wqz_ps = psum_st.tile([10, BC], F32, tag="stps", name="wqz")
        nc.tensor.matmul(wqz_ps, eye10_sb, sims, start=True, stop=False)
        nc.tensor.matmul(wqz_ps, mones10_sb, lnden, start=False, stop=True)
        wq0 = smalls.tile([10, BC], MID)
        nc.scalar.activation(wq0, wqz_ps, AF.Exp, bias=ln05)
        wq_sb = smalls.tile([10, BC], MID)
        nc.vector.tensor_mul(wq_sb, mask10, wq0)

        # fpw = softmax(z + wg_b) over the 5 encoders
        ez = smalls.tile([5, BC], MID)
        nc.scalar.activation(ez, z_ps, AF.Exp, bias=wgb_sb)
        sez_ps = psum_st.tile([1, BC], F32, tag="stps", name="sez")
        nc.tensor.matmul(sez_ps, ones_col16[0:5, :], ez, start=True, stop=True)
        lnsez = smalls.tile([1, BC], MID)
        nc.scalar.activation(lnsez, sez_ps, AF.Ln)
        zc = smalls.tile([5, BC], MID)
        nc.scalar.activation(zc, z_ps, AF.Copy)
        fz_ps = psum_st.tile([5, BC], F32, tag="stps", name="fz")
        nc.tensor.matmul(fz_ps, eye10_sb[0:5, 0:5], zc, start=True, stop=False)
        nc.tensor.matmul(fz_ps, mones10_sb[:, 0:5], lnsez, start=False, stop=True)
        fpw_sb = smalls.tile([5, BC], MID)
        nc.scalar.activation(fpw_sb, fz_ps, AF.Exp, bias=wgb_sb)

        def broadcast(dst, src_tile, row, nm):
            ksel = src_tile.shape[0]
            bc_ps = psum_bc.tile([P, BC], F32, tag="bcps", name=nm)
            nc.tensor.matmul(
                bc_ps,
                esel_sb[0:ksel, row * P : (row + 1) * P],
                src_tile,
                start=True,
                stop=True,
            )
            nc.scalar.activation(dst, bc_ps, AF.Copy)

        for p in range(10):
            broadcast(wqrep[:, p, :], wq_sb, p, f"bc_wq{p}")
        for i in range(5):
            broadcast(fpwrep[:, i, :], fpw_sb, i, f"bc_fpw{i}")
        broadcast(mfallrep, mfr, 0, "bc_mf")

        # ================= Phase C: masked aggregation (wide fp16) =========
        fps_by_ht = fps16.rearrange("p (i h) n -> p h i n", h=4)
        ew_t = w_pool.tile([P, 4, 512], FP16, tag="w16", name="ew_t")
        nc.sync.dma_start(ew_t, enh_w.ap().rearrange("(ko p) m -> p ko m", p=P))
        fw_view = fus_w.ap().rearrange("(ko p) m -> p ko m", p=P)
        fw_lo = w_pool.tile([P, 4, 512], FP16, tag="w16", name="fw_lo")
        nc.sync.dma_start(fw_lo, fw_view[:, 0:4, :])
        fw_hi = w_pool.tile([P, 4, 512], FP16, tag="w16", name="fw_hi")
        nc.sync.dma_start(fw_hi, fw_view[:, 4:8, :])

        enh_ps = [
            psum_mm.tile([P, BC], F32, tag="mmps", name=f"enh_{m}") for m in range(4)
        ]
        for ht in range(4):
            # common[ht] = sum_p mcf[p,ht] * wqrep[p] + mfall * meansum[ht]
            mcfwq = late_pool.tile([P, 10, BC], MID, tag="w10", name=f"mcfwq{ht}")
            nc.vector.tensor_mul(mcfwq, mcfw[:, :, ht, :], wqrep)
            t1 = late_pool.tile([P, 5, BC], MID, tag="t5", name=f"t1_{ht}")
            nc.vector.tensor_add(t1, mcfwq[:, 0:5, :], mcfwq[:, 5:10, :])
            t2 = late_pool.tile([P, 2, BC], MID, tag="t2", name=f"t2_{ht}")
            nc.vector.tensor_add(t2, t1[:, 0:2, :], t1[:, 2:4, :])
            mfm = late_pool.tile([P, BC], MID, tag="mfm", name=f"mfm_{ht}")
            nc.gpsimd.tensor_mul(mfm, meansum[:, ht, :], mfallrep)
            t3 = late_pool.tile([P, BC], MID, tag="t3", name=f"t3_{ht}")
            nc.vector.tensor_add(t3, t2[:, 0, :], t2[:, 1, :])
            t4 = late_pool.tile([P, BC], MID, tag="t4", name=f"t4_{ht}")
            nc.vector.tensor_add(t4, t3, t1[:, 4, :])
            nc.vector.tensor_add(common[:, ht, :], t4, mfm)
            # enhance matmul accumulates as soon as common[ht] is ready
            for m in range(4):
                nc.tensor.matmul(
                    enh_ps[m],
                    ew_t[:, ht, m * P : (m + 1) * P],
                    common[:, ht, :],
                    start=(ht == 0),
                    stop=(ht == 3),
                )
            # wsum[ht] = sum_i fps[i,ht] * fpwrep[i]  (Pool for 2 hts)
            weng = nc.gpsimd if ht % 2 == 1 else nc.vector
            uw = late_pool.tile([P, 5, BC], MID, tag="uw5", name=f"uw_{ht}")
            weng.tensor_mul(uw, fps_by_ht[:, ht, :, :], fpwrep)
            u1 = late_pool.tile([P, 2, BC], MID, tag="u2", name=f"u1_{ht}")
            weng.tensor_add(u1, uw[:, 0:2, :], uw[:, 2:4, :])
            u2 = late_pool.tile([P, BC], MID, tag="u1w", name=f"u2_{ht}")
            weng.tensor_add(u2, u1[:, 0, :], u1[:, 1, :])
            weng.tensor_add(wsum[:, ht, :], u2, uw[:, 4, :])

        # ================= Phase D: enhance gate + fuse =================
        for m in range(4):
            gate = gate_pool.tile([P, BC], MID, tag="gate", name=f"gate{m}")
            nc.scalar.activation(gate, enh_ps[m], AF.Sigmoid, bias=enhb_sb[:, m : m + 1])
            nc.vector.tensor_mul(enh_sb[:, m, :], common[:, m, :], gate)

        out_view = out.ap().rearrange("(m p) n -> p m n", p=P)
        for m in range(4):
            ps = psum_mm.tile([P, BC], F32, tag="mmps", name=f"fus_{m}")
            for k in range(8):
                rhs = wsum[:, k, :] if k < 4 else enh_sb[:, k - 4, :]
                fw_t = fw_lo if k < 4 else fw_hi
                nc.tensor.matmul(
                    ps,
                    fw_t[:, k % 4, m * P : (m + 1) * P],
                    rhs,
                    start=(k == 0),
                    stop=(k == 7),
                )
            o_sb = gate_pool.tile([P, BC], F32, tag="osb", name=f"osb{m}")
            nc.scalar.activation(o_sb, ps, AF.Identity, bias=fusb_sb[:, m : m + 1])
            nc.sync.dma_start(out_view[:, m, :], o_sb)


def prep_inputs(inputs):
    """Host-side: build the per-core in_maps from full inputs."""
    x = np.asarray(inputs["fp_features"], np.float32)

    def pad_rows(a, rows):
        a = np.asarray(a, np.float32)
        if a.shape[0] == rows:
            return a.astype(np.float16)
        out = np.zeros((rows, a.shape[1]), np.float16)
        out[: a.shape[0]] = a.astype(np.float16)
        return out

    # padded transposed x (fp16), shared prep then per-core column slices
    xt_full = np.zeros((XT_K * P, B), np.float16)
    offs_in = np.cumsum([0, AP_D, MA_D, MB_D, MC_D])
    for ei, (name, din, K, dh) in enumerate(ENCS):
        seg = x[:, offs_in[ei] : offs_in[ei] + din]  # [B, din]
        xt_full[XT_OFF[ei] * P : XT_OFF[ei] * P + din, :] = np.ascontiguousarray(
            seg.T
        ).astype(np.float16)

    common_map = {}
    for ei, (name, din, K, dh) in enumerate(ENCS):
        common_map[f"w1_{name}"] = pad_rows(inputs[f"{name}_w1"], K * P)
        common_map[f"w2_{name}"] = np.asarray(inputs[f"{name}_w2"], np.float16)
        common_map[f"b1_{name}"] = (
            np.asarray(inputs[f"{name}_b1"], np.float32).reshape(dh // P, P).T.copy()
        )
        common_map[f"b2_{name}"] = (
            np.asarray(inputs[f"{name}_b2"], np.float32).reshape(4, P).T.copy()
        )
    common_map["wg_w"] = np.asarray(inputs["wg_w"], np.float32).astype(np.float16)
    common_map["wg_b"] = np.asarray(inputs["wg_b"], np.float32).reshape(5, 1)
    pcat = np.zeros((5, 10), np.float16)
    for p in range(10):
        pcat[_I[p], p] = 1.0
        pcat[_J[p], p] = 1.0
    common_map["pcat"] = pcat
    esel = np.zeros((10, 10 * 128), np.float16)
    for p in range(10):
        esel[p, p * 128 : (p + 1) * 128] = 1.0
    common_map["esel"] = esel
    common_map["eye10"] = np.eye(10, dtype=np.float16)
    common_map["mones10"] = np.full((1, 10), -1.0, np.float16)
    common_map["enh_w"] = np.asarray(inputs["enh_w"], np.float16)
    common_map["enh_b"] = np.asarray(inputs["enh_b"], np.float32).reshape(4, P).T.copy()
    common_map["fus_w"] = np.asarray(inputs["fus_w"], np.float16)
    common_map["fus_b"] = np.asarray(inputs["fus_b"], np.float32).reshape(4, P).T.copy()

    in_maps = []
    for c in range(N_CORES):
        m = dict(common_map)
        m["xt"] = np.ascontiguousarray(xt_full[:, c * BC : (c + 1) * BC])
        in_maps.append(m)
    return in_maps


_NC_CACHE = None


def kernel(**inputs) -> np.ndarray:
    global _NC_CACHE
    if _NC_CACHE is None:
        _NC_CACHE = build_bass()
    nc = _NC_CACHE
    in_maps = prep_inputs(inputs)
    res = run_bass_kernel_spmd(nc, in_maps, core_ids=list(range(N_CORES)))
    outs = [res.results[c]["out"] for c in range(N_CORES)]  # each [H, BC]
    full = np.concatenate([o.T for o in outs], axis=0)  # [B, H]
    return np.ascontiguousarray(full.astype(np.float32))


# revision 16
# speedup vs baseline: 2.2731x; 1.0063x over previous
"""Trainium2 Bass kernel for nn_CommonFeatureExtractor (v2).

Data-parallel over 8 NeuronCores: batch dim (4096) sharded into 8 x 512,
weights replicated. Everything is computed in the transposed layout
[feature_on_partitions, batch_free].

v2 changes vs baseline:
  * all matmuls run in fp16 (fp32 LOW_HIGH pairs cost 4x on the PE)
  * pair products (fp16) are computed once on DVE/Pool during phase A and
    reused for both the d-statistics (ones-matmul partition reductions) and
    the masked pair features mcf_p = (prod_p>0)*(fps_I+fps_J), which are
    fully precomputed while the PE is busy with the encoder matmuls
  * softmax divisions are computed as exp(x - ln(denom)) where the
    subtraction is accumulated on the PE into PSUM (identity / -ones
    matmuls), avoiding the slow DVE reciprocal
  * phase C is a short chain of wide fp16 DVE ops: common = sum_p mcf_p *
    wqrep_p + fallback, interleaved with the enhance matmuls
"""

import numpy as np

import concourse.bass as bass
import concourse.mybir as mybir
import concourse.tile as tile
from concourse import bacc
from concourse.bass_utils import run_bass_kernel_spmd

F32 = mybir.dt.float32
FP16 = mybir.dt.float16
ALU = mybir.AluOpType
AF = mybir.ActivationFunctionType

N_CORES = 8
B = 4096
BC = B // N_CORES  # 512 samples per core
H = 512
P = 128

AP_D, MA_D, MB_D, MC_D, PH_D = 2048, 167, 2048, 2048, 27
# encoders: (name, din, padded K tiles, hidden dh)
ENCS = [
    ("ap", AP_D, 16, 512),
    ("ma", MA_D, 2, 256),
    ("mb", MB_D, 16, 512),
    ("mc", MC_D, 16, 512),
    ("ph", PH_D, 1, 128),
]
XT_K = sum(e[2] for e in ENCS)  # 51 padded k-tiles of x
XT_OFF = np.cumsum([0] + [e[2] for e in ENCS])[:-1]  # [0,16,18,34,50]

_I = [0, 0, 0, 0, 1, 1, 1, 2, 2, 3]
_J = [1, 2, 3, 4, 2, 3, 4, 3, 4, 4]
PAIR_IDX = {(_I[p], _J[p]): p for p in range(10)}
# compute order: small encoders first so most pair work overlaps phase A
ORDER = ["ma", "ph", "ap", "mb", "mc"]
ENC_BY_NAME = {e[0]: (i, e) for i, e in enumerate(ENCS)}
LAST_EI = ENC_BY_NAME[ORDER[-1]][0]

MID = FP16
LN_HALF = float(np.log(0.5))


def build_bass():
    nc = bacc.Bacc("TRN2", target_bir_lowering=False, debug=False)

    # ---------------- DRAM I/O ----------------
    xt = nc.dram_tensor("xt", [XT_K * P, BC], FP16, kind="ExternalInput")
    w1 = {}
    w2 = {}
    b1 = {}
    b2 = {}
    for name, _, K, dh in ENCS:
        w1[name] = nc.dram_tensor(f"w1_{name}", [K * P, dh], FP16, kind="ExternalInput")
        w2[name] = nc.dram_tensor(f"w2_{name}", [dh, H], FP16, kind="ExternalInput")
        b1[name] = nc.dram_tensor(f"b1_{name}", [P, dh // P], F32, kind="ExternalInput")
        b2[name] = nc.dram_tensor(f"b2_{name}", [P, 4], F32, kind="ExternalInput")
    wg_w = nc.dram_tensor("wg_w", [P, 20 * 5], FP16, kind="ExternalInput")
    wg_b = nc.dram_tensor("wg_b", [5, 1], F32, kind="ExternalInput")
    pcat = nc.dram_tensor("pcat", [5, 10], FP16, kind="ExternalInput")
    esel = nc.dram_tensor("esel", [10, 10 * P], FP16, kind="ExternalInput")
    eye10 = nc.dram_tensor("eye10", [10, 10], FP16, kind="ExternalInput")
    mones10 = nc.dram_tensor("mones10", [1, 10], FP16, kind="ExternalInput")
    enh_w = nc.dram_tensor("enh_w", [H, H], FP16, kind="ExternalInput")
    enh_b = nc.dram_tensor("enh_b", [P, 4], F32, kind="ExternalInput")
    fus_w = nc.dram_tensor("fus_w", [2 * H, H], FP16, kind="ExternalInput")
    fus_b = nc.dram_tensor("fus_b", [P, 4], F32, kind="ExternalInput")
    out = nc.dram_tensor("out", [H, BC], F32, kind="ExternalOutput")

    with tile.TileContext(nc) as tc:
        kernel_body(
            tc, xt, w1, w2, b1, b2, wg_w, wg_b, pcat, esel, eye10, mones10,
            enh_w, enh_b, fus_w, fus_b, out,
        )
    nc.compile()
    return nc


def kernel_body(
    tc, xt, w1, w2, b1, b2, wg_w, wg_b, pcat, esel, eye10, mones10,
    enh_w, enh_b, fus_w, fus_b, out,
):
    nc = tc.nc

    import contextlib

    ctx = contextlib.ExitStack()
    with ctx:
        # -------- pools --------
        persist = ctx.enter_context(tc.tile_pool(name="persist", bufs=1))
        smalls = ctx.enter_context(tc.tile_pool(name="smalls", bufs=1))
        statrows = ctx.enter_context(tc.tile_pool(name="statrows", bufs=2))
        psum_mm = ctx.enter_context(tc.tile_pool(name="psum_mm", bufs=4, space="PSUM"))
        psum_st = ctx.enter_context(tc.tile_pool(name="psum_st", bufs=2, space="PSUM"))
        psum_z = ctx.enter_context(tc.tile_pool(name="psum_z", bufs=1, space="PSUM"))
        psum_bc = ctx.enter_context(tc.tile_pool(name="psum_bc", bufs=1, space="PSUM"))
        pair_pool = ctx.enter_context(tc.tile_pool(name="pairp", bufs=2))
        late_pool = ctx.enter_context(tc.tile_pool(name="latep", bufs=1))
        w10_pool = ctx.enter_context(tc.tile_pool(name="w10p", bufs=1))
        xt_pool = ctx.enter_context(tc.tile_pool(name="xtp", bufs=2))
        w_pool = ctx.enter_context(tc.tile_pool(name="wp", bufs=2))
        h_pool = ctx.enter_context(tc.tile_pool(name="hp", bufs=2))
        gate_pool = ctx.enter_context(tc.tile_pool(name="gatep", bufs=2))

        # -------- persistent tiles --------
        fps16 = persist.tile([P, 20, BC], MID)  # fps.T, ktile = enc*4 + ht
        mcfw = persist.tile([P, 10, 4, BC], MID)  # masked pair features
        wqrep = persist.tile([P, 10, BC], MID)
        fpwrep = persist.tile([P, 5, BC], MID)
        mfallrep = persist.tile([P, BC], MID)
        mpartA = persist.tile([P, 4, BC], MID)  # mean partial (ping)
        meansum = persist.tile([P, 4, BC], MID)  # sum of all 5 fps (pong)
        common = persist.tile([P, 4, BC], MID)
        wsum = persist.tile([P, 4, BC], MID)
        enh_sb = persist.tile([P, 4, BC], MID)
        stats = persist.tile([10, BC], MID)  # pair dots d
        ss_sb = persist.tile([5, BC], MID)  # squared norms
        ones_col16 = persist.tile([P, 1], MID)
        ln05 = persist.tile([10, 1], F32)
        pcat_sb = persist.tile([5, 10], MID)
        esel_sb = persist.tile([10, 10 * P], MID)
        eye10_sb = persist.tile([10, 10], MID)
        mones10_sb = persist.tile([1, 10], MID)
        biases = {}
        for name, _, K, dh in ENCS:
            biases[name] = (
                persist.tile([P, dh // P], F32, name=f"b1sb_{name}"),
                persist.tile([P, 4], F32, name=f"b2sb_{name}"),
            )
        wgb_sb = persist.tile([5, 1], F32)
        enhb_sb = persist.tile([P, 4], F32)
        fusb_sb = persist.tile([P, 4], F32)
        wg_sb = persist.tile([P, 20, 5], FP16)

        nc.vector.memset(ones_col16, 1.0)
        nc.vector.memset(ln05, LN_HALF)
        nc.gpsimd.dma_start(pcat_sb, pcat.ap())
        nc.gpsimd.dma_start(esel_sb, esel.ap())
        nc.gpsimd.dma_start(eye10_sb, eye10.ap())
        nc.gpsimd.dma_start(mones10_sb, mones10.ap())
        for name, _, K, dh in ENCS:
            nc.gpsimd.dma_start(biases[name][0], b1[name].ap())
            nc.gpsimd.dma_start(biases[name][1], b2[name].ap())
        nc.gpsimd.dma_start(wgb_sb, wg_b.ap())
        nc.gpsimd.dma_start(enhb_sb, enh_b.ap())
        nc.gpsimd.dma_start(fusb_sb, fus_b.ap())
        nc.gpsimd.dma_start(wg_sb, wg_w.ap().rearrange("p (ko m) -> p ko m", m=5))

        xt_view = xt.ap().rearrange("(ko p) n -> p ko n", p=P)

        # z-gate psum, accumulated across all 20 fps tiles during phase A
        z_ps = psum_z.tile([5, BC], F32, tag="zps", name="zgate")
        z_idx = [0]

        def stat_row_to(dst, row, ps, nm):
            srow = statrows.tile([1, BC], MID, tag="statrow", name=f"srow_{nm}")
            nc.scalar.activation(srow, ps, AF.Copy)
            nc.gpsimd.dma_start(dst[row : row + 1, :], srow)

        def emit_d_group(p, prod_wide, nm):
            # d_p = sum_h prod over all 4 ht tiles (ones-matmul reduction)
            ps = psum_st.tile([1, BC], F32, tag="stps", name=f"d_{nm}")
            for ht in range(4):
                nc.tensor.matmul(
                    ps, ones_col16, prod_wide[:, ht, :], start=(ht == 0), stop=(ht == 3)
                )
            stat_row_to(stats, p, ps, f"d{p}")

        def emit_pair(pi, pj):
            # prod (DVE, feeds d-matmuls fast), s on Pool in parallel,
            # mask+mcf on DVE (TS+TT run at 4x; STT only runs at 1x)
            p = PAIR_IDX[(min(pi, pj), max(pi, pj))]
            fi = fps16[:, pi * 4 : pi * 4 + 4, :]
            fj = fps16[:, pj * 4 : pj * 4 + 4, :]
            prod = pair_pool.tile([P, 4, BC], MID, tag="prod", name=f"prod_{p}")
            nc.vector.tensor_mul(prod, fi, fj)
            s_t = pair_pool.tile([P, 4, BC], MID, tag="s", name=f"s_{p}")
            nc.gpsimd.tensor_add(s_t, fi, fj)
            nc.vector.scalar_tensor_tensor(
                mcfw[:, p, :, :], in0=prod, scalar=0.0, in1=s_t,
                op0=ALU.is_gt, op1=ALU.mult,
            )
            emit_d_group(p, prod, f"p{p}")

        def emit_ss(ei):
            sq = pair_pool.tile([P, 4, BC], MID, tag="s", name=f"sq_{ei}")
            nc.scalar.square(sq, fps16[:, ei * 4 : ei * 4 + 4, :])
            ps = psum_st.tile([1, BC], F32, tag="stps", name=f"ss_{ei}")
            for ht in range(4):
                nc.tensor.matmul(
                    ps, ones_col16, sq[:, ht, :], start=(ht == 0), stop=(ht == 3)
                )
            stat_row_to(ss_sb, ei, ps, f"ss{ei}")

        # ================= Phase A: encoders + interleaved pair work =======
        done_encs = []
        pair_count = [0]
        for name in ORDER:
            ei, (_, _, K, dh) = ENC_BY_NAME[name]
            M = dh // P
            b1_sb, b2_sb = biases[name]
            # ---- layer 1: h.T[dh, BC] = relu(w1.T @ x.T + b1) ----
            psums = [
                psum_mm.tile([P, BC], F32, tag="mmps", name=f"l1_{name}_{m}")
                for m in range(M)
            ]
            h_sb = h_pool.tile([P, 4, BC], MID, tag="htile")
            kdone = 0
            for kc0 in range(0, K, 4):
                kn = min(4, K - kc0)
                xt_t = xt_pool.tile([P, 4, BC], FP16, tag="xt")
                nc.sync.dma_start(
                    xt_t[:, :kn, :],
                    xt_view[:, XT_OFF[ei] + kc0 : XT_OFF[ei] + kc0 + kn, :],
                )
                w1_t = w_pool.tile([P, 4, 512], FP16, tag="w1")
                nc.sync.dma_start(
                    w1_t[:, :kn, :dh],
                    w1[name].ap()[kc0 * P : (kc0 + kn) * P, :].rearrange(
                        "(ko p) m -> p ko m", p=P
                    ),
                )
                for m in range(M):
                    for k in range(kn):
                        nc.tensor.matmul(
                            psums[m],
                            w1_t[:, k, m * P : (m + 1) * P],
                            xt_t[:, k, :],
                            start=(kdone + k == 0),
                            stop=(kdone + k == K - 1),
                        )
                kdone += kn
            for m in range(M):
                nc.scalar.activation(
                    h_sb[:, m, :], psums[m], AF.Relu, bias=b1_sb[:, m : m + 1]
                )
            # ---- layer 2: fps.T[H, BC] = w2.T @ h.T + b2 ----
            w2_t = w_pool.tile([P, 4, 512], FP16, tag="w1", name=f"w2_{name}")
            nc.sync.dma_start(
                w2_t[:, :M, :], w2[name].ap().rearrange("(ko p) m -> p ko m", p=P)
            )
            for m in range(4):
                ps = psum_mm.tile([P, BC], F32, tag="mmps", name=f"l2_{name}_{m}")
                for k in range(M):
                    nc.tensor.matmul(
                        ps,
                        w2_t[:, k, m * P : (m + 1) * P],
                        h_sb[:, k, :],
                        start=(k == 0),
                        stop=(k == M - 1),
                    )
                nc.scalar.activation(
                    fps16[:, ei * 4 + m, :], ps, AF.Identity, bias=b2_sb[:, m : m + 1]
                )
                # z-gate accumulation for the learned fusion weights
                nc.tensor.matmul(
                    z_ps,
                    wg_sb[:, ei * 4 + m, :],
                    fps16[:, ei * 4 + m, :],
                    start=(z_idx[0] == 0),
                    stop=(z_idx[0] == 19),
                )
                z_idx[0] += 1
            # ---- interleaved stats ----
            emit_ss(ei)
            if name != ORDER[-1]:
                for prev in done_encs:
                    emit_pair(prev, ei)
                    pair_count[0] += 1
                # partial mean-fallback sum (first 4 encoders), ping-pong
                if len(done_encs) == 1:
                    nc.vector.tensor_add(
                        mpartA,
                        fps16[:, done_encs[0] * 4 : done_encs[0] * 4 + 4, :],
                        fps16[:, ei * 4 : ei * 4 + 4, :],
                    )
                elif len(done_encs) == 2:
                    nc.vector.tensor_add(
                        meansum, mpartA, fps16[:, ei * 4 : ei * 4 + 4, :]
                    )
                elif len(done_encs) == 3:
                    nc.vector.tensor_add(
                        mpartA, meansum, fps16[:, ei * 4 : ei * 4 + 4, :]
                    )
            done_encs.append(ei)

        # ---- everything below needs only ss / z / mc-fps: run it while ----
        # ---- the tail pair work (DVE/Pool) streams in parallel          ----
        last = LAST_EI
        # total mean-fallback sum
        nc.vector.tensor_add(meansum, mpartA, fps16[:, last * 4 : last * 4 + 4, :])
        # tail pairs (prod on DVE feeds the d-matmuls quickly; s on Pool)
        for prev in done_encs[:-1]:
            emit_pair(prev, last)

        # ================= Phase B: pair softmax + fusion gate =============
        # l5 = ln(ss); pl = pcat @ l5; invnn = exp(-0.5*pl)   (no d needed)
        l5 = smalls.tile([5, BC], MID)
        nc.scalar.activation(l5, ss_sb, AF.Ln)
        pl_ps = psum_st.tile([10, BC], F32, tag="stps", name="pl")
        nc.tensor.matmul(pl_ps, pcat_sb, l5, start=True, stop=True)
        invnn = smalls.tile([10, BC], MID)
        nc.scalar.activation(invnn, pl_ps, AF.Exp, scale=-0.5)

        # fpw = softmax(z + wg_b) over the 5 encoders  (no d needed)
        ez = smalls.tile([5, BC], MID)
        nc.scalar.activation(ez, z_ps, AF.Exp, bias=wgb_sb)
        sez_ps = psum_st.tile([1, BC], F32, tag="stps", name="sez")
        nc.tensor.matmul(sez_ps, ones_col16[0:5, :], ez, start=True, stop=True)
        lnsez = smalls.tile([1, BC], MID)
        nc.scalar.activation(lnsez, sez_ps, AF.Ln)
        zc = smalls.tile([5, BC], MID)
        nc.scalar.activation(zc, z_ps, AF.Copy)
        fz_ps = psum_st.tile([5, BC], F32, tag="stps", name="fz")
        nc.tensor.matmul(fz_ps, eye10_sb[0:5, 0:5], zc, start=True, stop=False)
        nc.tensor.matmul(fz_ps, mones10_sb[:, 0:5], lnsez, start=False, stop=True)
        fpw_sb = smalls.tile([5, BC], MID)
        nc.scalar.activation(fpw_sb, fz_ps, AF.Exp, bias=wgb_sb)

        bc_idx = [0]

        def broadcast(dst, src_tile, row, nm):
            ksel = src_tile.shape[0]
            pool = [psum_bc, psum_st, psum_st][bc_idx[0] % 3]
            bc_idx[0] += 1
            bc_ps = pool.tile(
                [P, BC], F32, tag="bcps" if pool is psum_bc else "stps", name=nm
            )
            nc.tensor.matmul(
                bc_ps,
                esel_sb[0:ksel, row * P : (row + 1) * P],
                src_tile,
                start=True,
                stop=True,
            )
            nc.scalar.activation(dst, bc_ps, AF.Copy)

        for i in range(5):
            broadcast(fpwrep[:, i, :], fpw_sb, i, f"bc_fpw{i}")

        # wsum[ht] = sum_i fps[i,ht] * fpwrep[i] — only needs fpwrep, so it
        # runs here, fully overlapped with the wq chain below
        fps_by_ht = fps16.rearrange("p (i h) n -> p h i n", h=4)
        for ht in range(4):
            weng = nc.gpsimd if ht % 2 == 1 else nc.vector
            uw = late_pool.tile([P, 5, BC], MID, tag="uw5", name=f"uw_{ht}")
            weng.tensor_mul(uw, fps_by_ht[:, ht, :, :], fpwrep)
            u1 = late_pool.tile([P, 2, BC], MID, tag="u2", name=f"u1_{ht}")
            weng.tensor_add(u1, uw[:, 0:2, :], uw[:, 2:4, :])
            u2 = late_pool.tile([P, BC], MID, tag="u1w", name=f"u2_{ht}")
            weng.tensor_add(u2, u1[:, 0, :], u1[:, 1, :])
            weng.tensor_add(wsum[:, ht, :], u2, uw[:, 4, :])

        # wq chain (needs the tail-pair d stats)
        sims = smalls.tile([10, BC], MID)
        nc.vector.tensor_mul(sims, stats[0:10, :], invnn)
        mask10 = smalls.tile([10, BC], MID)
        nc.vector.tensor_scalar(
            mask10, in0=stats[0:10, :], scalar1=0.0, scalar2=None, op0=ALU.is_gt
        )
        e0 = smalls.tile([10, BC], MID)
        nc.scalar.activation(e0, sims, AF.Exp)
        e_sb = smalls.tile([10, BC], MID)
        nc.vector.tensor_mul(e_sb, mask10, e0)
        den_ps = psum_st.tile([1, BC], F32, tag="stps", name="den")
        nc.tensor.matmul(den_ps, ones_col16[0:10, :], e_sb, start=True, stop=True)
        # mean-fallback row: 0.2 iff no pair selected
        mfr = smalls.tile([1, BC], MID)
        nc.vector.tensor_scalar(
            mfr, in0=den_ps, scalar1=0.0, scalar2=0.2, op0=ALU.is_le, op1=ALU.mult
        )
        # wq = 0.5 * mask * exp(sims - ln(max(den,1)))
        den_c = smalls.tile([1, BC], F32)
        nc.vector.tensor_scalar_max(den_c, den_ps, 1.0)
        lnden = smalls.tile([1, BC], MID)
        nc.scalar.activation(lnden, den_c, AF.Ln)
        wqz_ps = psum_st.tile([10, BC], F32, tag="stps", name="wqz")
        nc.tensor.matmul(wqz_ps, eye10_sb, sims, start=True, stop=False)
        nc.tensor.matmul(wqz_ps, mones10_sb, lnden, start=False, stop=True)
        wq0 = smalls.tile([10, BC], MID)
        nc.scalar.activation(wq0, wqz_ps, AF.Exp, bias=ln05)
        wq_sb = smalls.tile([10, BC], MID)
        nc.vector.tensor_mul(wq_sb, mask10, wq0)

        broadcast(mfallrep, mfr, 0, "bc_mf")
        for p in range(10):
            broadcast(wqrep[:, p, :], wq_sb, p, f"bc_wq{p}")

        # ================= Phase C: masked aggregation (wide fp16) =========
        ew_t = w_pool.tile([P, 4, 512], FP16, tag="w16", name="ew_t")
        nc.sync.dma_start(ew_t, enh_w.ap().rearrange("(ko p) m -> p ko m", p=P))
        fw_view = fus_w.ap().rearrange("(ko p) m -> p ko m", p=P)
        fw_lo = w_pool.tile([P, 4, 512], FP16, tag="w16", name="fw_lo")
        nc.sync.dma_start(fw_lo, fw_view[:, 0:4, :])
        fw_hi = w_pool.tile([P, 4, 512], FP16, tag="w16", name="fw_hi")
        nc.sync.dma_start(fw_hi, fw_view[:, 4:8, :])

        enh_ps = [
            psum_mm.tile([P, BC], F32, tag="mmps", name=f"enh_{m}") for m in range(4)
        ]
        for ht in range(4):
            # common[ht] = sum_p mcf[p,ht] * wqrep[p] + mfall * meansum[ht]
            mcfwq = w10_pool.tile([P, 10, BC], MID, tag="w10", name=f"mcfwq{ht}")
            nc.vector.tensor_mul(mcfwq, mcfw[:, :, ht, :], wqrep)
            t1 = late_pool.tile([P, 5, BC], MID, tag="t5", name=f"t1_{ht}")
            nc.vector.tensor_add(t1, mcfwq[:, 0:5, :], mcfwq[:, 5:10, :])
            t2 = late_pool.tile([P, 2, BC], MID, tag="t2", name=f"t2_{ht}")
            nc.vector.tensor_add(t2, t1[:, 0:2, :], t1[:, 2:4, :])
            mfm = late_pool.tile([P, BC], MID, tag="mfm", name=f"mfm_{ht}")
            nc.gpsimd.tensor_mul(mfm, meansum[:, ht, :], mfallrep)
            t3 = late_pool.tile([P, BC], MID, tag="t3", name=f"t3_{ht}")
            nc.vector.tensor_add(t3, t2[:, 0, :], t2[:, 1, :])
            t4 = late_pool.tile([P, BC], MID, tag="t4", name=f"t4_{ht}")
            nc.vector.tensor_add(t4, t3, t1[:, 4, :])
            nc.vector.tensor_add(common[:, ht, :], t4, mfm)
            # enhance matmul accumulates as soon as common[ht] is ready
            for m in range(4):
                nc.tensor.matmul(
                    enh_ps[m],
                    ew_t[:, ht, m * P : (m + 1) * P],
                    common[:, ht, :],
                    start=(ht == 0),
                    stop=(ht == 3),
                )

        # ================= Phase D: enhance gate + fuse =================
        for m in range(4):
            gate = gate_pool.tile([P, BC], MID, tag="gate", name=f"gate{m}")
            nc.scalar.activation(gate, enh_ps[m], AF.Sigmoid, bias=enhb_sb[:, m : m + 1])
            nc.vector.tensor_mul(enh_sb[:, m, :], common[:, m, :], gate)

        out_view = out.ap().rearrange("(m p) n -> p m n", p=P)
        for m in range(4):
            ps = psum_mm.tile([P, BC], F32, tag="mmps", name=f"fus_{m}")
            for k in range(8):
                rhs = wsum[:, k, :] if k < 4 else enh_sb[:, k - 4, :]
                fw_t = fw_lo if k < 4 else fw_hi
                nc.tensor.matmul(
                    ps,
                    fw_t[:, k % 4, m * P : (m + 1) * P],
                    rhs,
                    start=(k == 0),
                    stop=(k == 7),
                )
            o_sb = gate_pool.tile([P, BC], F32, tag="osb", name=f"osb{m}")
            nc.scalar.activation(o_sb, ps, AF.Identity, bias=fusb_sb[:, m : m + 1])
            nc.sync.dma_start(out_view[:, m, :], o_sb)


def prep_inputs(inputs):
    """Host-side: build the per-core in_maps from full inputs."""
    x = np.asarray(inputs["fp_features"], np.float32)

    def pad_rows(a, rows):
        a = np.asarray(a, np.float32)
        if a.shape[0] == rows:
            return a.astype(np.float16)
        out = np.zeros((rows, a.shape[1]), np.float16)
        out[: a.shape[0]] = a.astype(np.float16)
        return out

    # padded transposed x (fp16), shared prep then per-core column slices
    xt_full = np.zeros((XT_K * P, B), np.float16)
    offs_in = np.cumsum([0, AP_D, MA_D, MB_D, MC_D])
    for ei, (name, din, K, dh) in enumerate(ENCS):
        seg = x[:, offs_in[ei] : offs_in[ei] + din]  # [B, din]
        xt_full[XT_OFF[ei] * P : XT_OFF[ei] * P + din, :] = np.ascontiguousarray(
            seg.T
        ).astype(np.float16)

    common_map = {}
    for ei, (name, din, K, dh) in enumerate(ENCS):
        common_map[f"w1_{name}"] = pad_rows(inputs[f"{name}_w1"], K * P)
        common_map[f"w2_{name}"] = np.asarray(inputs[f"{name}_w2"], np.float16)
        common_map[f"b1_{name}"] = (
            np.asarray(inputs[f"{name}_b1"], np.float32).reshape(dh // P, P).T.copy()
        )
        common_map[f"b2_{name}"] = (
            np.asarray(inputs[f"{name}_b2"], np.float32).reshape(4, P).T.copy()
        )
    wgw = np.asarray(inputs["wg_w"], np.float32).astype(np.float16)  # [2560, 5]
    common_map["wg_w"] = np.ascontiguousarray(
        wgw.reshape(20, 128, 5).transpose(1, 0, 2).reshape(128, 100))
    common_map["wg_b"] = np.asarray(inputs["wg_b"], np.float32).reshape(5, 1)
    pcat = np.zeros((5, 10), np.float16)
    for p in range(10):
        pcat[_I[p], p] = 1.0
        pcat[_J[p], p] = 1.0
    common_map["pcat"] = pcat
    esel = np.zeros((10, 10 * 128), np.float16)
    for p in range(10):
        esel[p, p * 128 : (p + 1) * 128] = 1.0
    common_map["esel"] = esel
    common_map["eye10"] = np.eye(10, dtype=np.float16)
    common_map["mones10"] = np.full((1, 10), -1.0, np.float16)
    common_map["enh_w"] = np.asarray(inputs["enh_w"], np.float16)
    common_map["enh_b"] = np.asarray(inputs["enh_b"], np.float32).reshape(4, P).T.copy()
    common_map["fus_w"] = np.asarray(inputs["fus_w"], np.float16)
    common_map["fus_b"] = np.asarray(inputs["fus_b"], np.float32).reshape(4, P).T.copy()

    in_maps = []
    for c in range(N_CORES):
        m = dict(common_map)
        m["xt"] = np.ascontiguousarray(xt_full[:, c * BC : (c + 1) * BC])
        in_maps.append(m)
    return in_maps


_NC_CACHE = None


def kernel(**inputs) -> np.ndarray:
    global _NC_CACHE
    if _NC_CACHE is None:
        _NC_CACHE = build_bass()
    nc = _NC_CACHE
    in_maps = prep_inputs(inputs)
    res = run_bass_kernel_spmd(nc, in_maps, core_ids=list(range(N_CORES)))
    outs = [res.results[c]["out"] for c in range(N_CORES)]  # each [H, BC]
    full = np.concatenate([o.T for o in outs], axis=0)  # [B, H]
    return np.ascontiguousarray(full.astype(np.float32))


# revision 17
# speedup vs baseline: 2.4324x; 1.0701x over previous
"""Trainium2 Bass kernel for nn_CommonFeatureExtractor (v2).

Data-parallel over 8 NeuronCores: batch dim (4096) sharded into 8 x 512,
weights replicated. Everything is computed in the transposed layout
[feature_on_partitions, batch_free].

v2 changes vs baseline:
  * all matmuls run in fp16 (fp32 LOW_HIGH pairs cost 4x on the PE)
  * pair products (fp16) are computed once on DVE/Pool during phase A and
    reused for both the d-statistics (ones-matmul partition reductions) and
    the masked pair features mcf_p = (prod_p>0)*(fps_I+fps_J), which are
    fully precomputed while the PE is busy with the encoder matmuls
  * softmax divisions are computed as exp(x - ln(denom)) where the
    subtraction is accumulated on the PE into PSUM (identity / -ones
    matmuls), avoiding the slow DVE reciprocal
  * phase C is a short chain of wide fp16 DVE ops: common = sum_p mcf_p *
    wqrep_p + fallback, interleaved with the enhance matmuls
"""

import numpy as np

import concourse.bass as bass
import concourse.mybir as mybir
import concourse.tile as tile
from concourse import bacc
from concourse.bass_utils import run_bass_kernel_spmd

F32 = mybir.dt.float32
FP16 = mybir.dt.float16
ALU = mybir.AluOpType
AF = mybir.ActivationFunctionType

N_CORES = 8
B = 4096
BC = B // N_CORES  # 512 samples per core
H = 512
P = 128

AP_D, MA_D, MB_D, MC_D, PH_D = 2048, 167, 2048, 2048, 27
# encoders: (name, din, padded K tiles, hidden dh)
ENCS = [
    ("ap", AP_D, 16, 512),
    ("ma", MA_D, 2, 256),
    ("mb", MB_D, 16, 512),
    ("mc", MC_D, 16, 512),
    ("ph", PH_D, 1, 128),
]
XT_K = sum(e[2] for e in ENCS)  # 51 padded k-tiles of x
XT_OFF = np.cumsum([0] + [e[2] for e in ENCS])[:-1]  # [0,16,18,34,50]

_I = [0, 0, 0, 0, 1, 1, 1, 2, 2, 3]
_J = [1, 2, 3, 4, 2, 3, 4, 3, 4, 4]
PAIR_IDX = {(_I[p], _J[p]): p for p in range(10)}
# compute order: small encoders first so most pair work overlaps phase A
ORDER = ["ma", "ph", "ap", "mb", "mc"]
ENC_BY_NAME = {e[0]: (i, e) for i, e in enumerate(ENCS)}
LAST_EI = ENC_BY_NAME[ORDER[-1]][0]

MID = FP16
LN_HALF = float(np.log(0.5))


def build_bass():
    nc = bacc.Bacc("TRN2", target_bir_lowering=False, debug=False)

    # ---------------- DRAM I/O ----------------
    xt = nc.dram_tensor("xt", [XT_K * P, BC], FP16, kind="ExternalInput")
    w1 = {}
    w2 = {}
    b1 = {}
    b2 = {}
    for name, _, K, dh in ENCS:
        w1[name] = nc.dram_tensor(f"w1_{name}", [K * P, dh], FP16, kind="ExternalInput")
        w2[name] = nc.dram_tensor(f"w2_{name}", [dh, H], FP16, kind="ExternalInput")
        b1[name] = nc.dram_tensor(f"b1_{name}", [P, dh // P], F32, kind="ExternalInput")
        b2[name] = nc.dram_tensor(f"b2_{name}", [P, 4], F32, kind="ExternalInput")
    wg_w = nc.dram_tensor("wg_w", [P, 20 * 5], FP16, kind="ExternalInput")
    wg_b = nc.dram_tensor("wg_b", [5, 1], F32, kind="ExternalInput")
    pcat = nc.dram_tensor("pcat", [5, 10], FP16, kind="ExternalInput")
    esel = nc.dram_tensor("esel", [10, 10 * P], FP16, kind="ExternalInput")
    eye10 = nc.dram_tensor("eye10", [10, 10], FP16, kind="ExternalInput")
    mones10 = nc.dram_tensor("mones10", [1, 10], FP16, kind="ExternalInput")
    enh_w = nc.dram_tensor("enh_w", [H, H], FP16, kind="ExternalInput")
    enh_b = nc.dram_tensor("enh_b", [P, 4], F32, kind="ExternalInput")
    fus_w = nc.dram_tensor("fus_w", [2 * H, H], FP16, kind="ExternalInput")
    fus_b = nc.dram_tensor("fus_b", [P, 4], F32, kind="ExternalInput")
    out = nc.dram_tensor("out", [H, BC], F32, kind="ExternalOutput")

    with tile.TileContext(nc) as tc:
        kernel_body(
            tc, xt, w1, w2, b1, b2, wg_w, wg_b, pcat, esel, eye10, mones10,
            enh_w, enh_b, fus_w, fus_b, out,
        )
    nc.compile()
    return nc


def kernel_body(
    tc, xt, w1, w2, b1, b2, wg_w, wg_b, pcat, esel, eye10, mones10,
    enh_w, enh_b, fus_w, fus_b, out,
):
    nc = tc.nc

    import contextlib

    ctx = contextlib.ExitStack()
    with ctx:
        # -------- pools --------
        persist = ctx.enter_context(tc.tile_pool(name="persist", bufs=1))
        smalls = ctx.enter_context(tc.tile_pool(name="smalls", bufs=1))
        statrows = ctx.enter_context(tc.tile_pool(name="statrows", bufs=2))
        psum_mm = ctx.enter_context(tc.tile_pool(name="psum_mm", bufs=4, space="PSUM"))
        psum_st = ctx.enter_context(tc.tile_pool(name="psum_st", bufs=2, space="PSUM"))
        psum_z = ctx.enter_context(tc.tile_pool(name="psum_z", bufs=1, space="PSUM"))
        psum_bc = ctx.enter_context(tc.tile_pool(name="psum_bc", bufs=1, space="PSUM"))
        pair_pool = ctx.enter_context(tc.tile_pool(name="pairp", bufs=2))
        late_pool = ctx.enter_context(tc.tile_pool(name="latep", bufs=1))
        w10_pool = ctx.enter_context(tc.tile_pool(name="w10p", bufs=1))
        xt_pool = ctx.enter_context(tc.tile_pool(name="xtp", bufs=2))
        w_pool = ctx.enter_context(tc.tile_pool(name="wp", bufs=2))
        h_pool = ctx.enter_context(tc.tile_pool(name="hp", bufs=2))
        gate_pool = ctx.enter_context(tc.tile_pool(name="gatep", bufs=2))

        # -------- persistent tiles --------
        fps16 = persist.tile([P, 20, BC], MID)  # fps.T, ktile = enc*4 + ht
        mcfw = persist.tile([P, 4, 10, BC], MID)  # masked pair features, ht-major
        wqrep = persist.tile([P, 10, BC], MID)
        fpwrep = persist.tile([P, 5, BC], MID)
        mfallrep = persist.tile([P, BC], MID)
        mpartA = persist.tile([P, 4, BC], MID)  # mean partial (ping)
        meansum = persist.tile([P, 4, BC], MID)  # sum of all 5 fps (pong)
        common = persist.tile([P, 4, BC], MID)
        wsum = persist.tile([P, 4, BC], MID)
        enh_sb = persist.tile([P, 4, BC], MID)
        stats = persist.tile([10, BC], MID)  # pair dots d
        ss_sb = persist.tile([5, BC], MID)  # squared norms
        ones_col16 = persist.tile([P, 1], MID)
        ln05 = persist.tile([10, 1], F32)
        pcat_sb = persist.tile([5, 10], MID)
        esel_sb = persist.tile([10, 10 * P], MID)
        eye10_sb = persist.tile([10, 10], MID)
        mones10_sb = persist.tile([1, 10], MID)
        biases = {}
        for name, _, K, dh in ENCS:
            biases[name] = (
                persist.tile([P, dh // P], F32, name=f"b1sb_{name}"),
                persist.tile([P, 4], F32, name=f"b2sb_{name}"),
            )
        wgb_sb = persist.tile([5, 1], F32)
        enhb_sb = persist.tile([P, 4], F32)
        fusb_sb = persist.tile([P, 4], F32)
        wg_sb = persist.tile([P, 20, 5], FP16)

        nc.vector.memset(ones_col16, 1.0)
        nc.vector.memset(ln05, LN_HALF)

        consts_emitted = [False]

        def emit_const_dmas():
            # emitted right after the first compute-critical chunk DMAs so
            # they queue behind them on the sync DMA queue
            if consts_emitted[0]:
                return
            consts_emitted[0] = True
            nc.sync.dma_start(pcat_sb, pcat.ap())
            nc.sync.dma_start(esel_sb, esel.ap())
            nc.sync.dma_start(eye10_sb, eye10.ap())
            nc.sync.dma_start(mones10_sb, mones10.ap())
            for name, _, K, dh in ENCS:
                nc.sync.dma_start(biases[name][0], b1[name].ap())
                nc.sync.dma_start(biases[name][1], b2[name].ap())
            nc.sync.dma_start(wgb_sb, wg_b.ap())
            nc.sync.dma_start(enhb_sb, enh_b.ap())
            nc.sync.dma_start(fusb_sb, fus_b.ap())
            nc.sync.dma_start(wg_sb, wg_w.ap().rearrange("p (ko m) -> p ko m", m=5))

        xt_view = xt.ap().rearrange("(ko p) n -> p ko n", p=P)

        # z-gate psum, accumulated across all 20 fps tiles during phase A
        z_ps = psum_z.tile([5, BC], F32, tag="zps", name="zgate")
        z_idx = [0]

        def stat_row_to(dst, row, ps, nm):
            srow = statrows.tile([1, BC], MID, tag="statrow", name=f"srow_{nm}")
            nc.scalar.activation(srow, ps, AF.Copy)
            nc.sync.dma_start(dst[row : row + 1, :], srow)

        def emit_d_group(p, prod_wide, nm):
            # d_p = sum_h prod over all 4 ht tiles (ones-matmul reduction)
            ps = psum_st.tile([1, BC], F32, tag="stps", name=f"d_{nm}")
            for ht in range(4):
                nc.tensor.matmul(
                    ps, ones_col16, prod_wide[:, ht, :], start=(ht == 0), stop=(ht == 3)
                )
            stat_row_to(stats, p, ps, f"d{p}")

        def emit_pair(pi, pj):
            # prod (DVE, feeds d-matmuls fast), s on Pool in parallel,
            # mask+mcf on DVE (TS+TT run at 4x; STT only runs at 1x)
            p = PAIR_IDX[(min(pi, pj), max(pi, pj))]
            fi = fps16[:, pi * 4 : pi * 4 + 4, :]
            fj = fps16[:, pj * 4 : pj * 4 + 4, :]
            prod = pair_pool.tile([P, 4, BC], MID, tag="prod", name=f"prod_{p}")
            nc.vector.tensor_mul(prod, fi, fj)
            s_t = pair_pool.tile([P, 4, BC], MID, tag="s", name=f"s_{p}")
            nc.vector.tensor_add(s_t, fi, fj)
            nc.vector.scalar_tensor_tensor(
                mcfw[:, :, p, :], in0=prod, scalar=0.0, in1=s_t,
                op0=ALU.is_gt, op1=ALU.mult,
            )
            emit_d_group(p, prod, f"p{p}")

        def emit_ss(ei):
            sq = pair_pool.tile([P, 4, BC], MID, tag="s", name=f"sq_{ei}")
            nc.scalar.square(sq, fps16[:, ei * 4 : ei * 4 + 4, :])
            ps = psum_st.tile([1, BC], F32, tag="stps", name=f"ss_{ei}")
            for ht in range(4):
                nc.tensor.matmul(
                    ps, ones_col16, sq[:, ht, :], start=(ht == 0), stop=(ht == 3)
                )
            stat_row_to(ss_sb, ei, ps, f"ss{ei}")

        # ================= Phase A: encoders + interleaved pair work =======
        done_encs = []
        pair_count = [0]
        for name in ORDER:
            ei, (_, _, K, dh) = ENC_BY_NAME[name]
            M = dh // P
            b1_sb, b2_sb = biases[name]
            # ---- layer 1: h.T[dh, BC] = relu(w1.T @ x.T + b1) ----
            psums = [
                psum_mm.tile([P, BC], F32, tag="mmps", name=f"l1_{name}_{m}")
                for m in range(M)
            ]
            h_sb = h_pool.tile([P, 4, BC], MID, tag="htile")
            kdone = 0
            for kc0 in range(0, K, 4):
                kn = min(4, K - kc0)
                xt_t = xt_pool.tile([P, 4, BC], FP16, tag="xt")
                nc.sync.dma_start(
                    xt_t[:, :kn, :],
                    xt_view[:, XT_OFF[ei] + kc0 : XT_OFF[ei] + kc0 + kn, :],
                )
                w1_t = w_pool.tile([P, 4, 512], FP16, tag="w1")
                nc.sync.dma_start(
                    w1_t[:, :kn, :dh],
                    w1[name].ap()[kc0 * P : (kc0 + kn) * P, :].rearrange(
                        "(ko p) m -> p ko m", p=P
                    ),
                )
                emit_const_dmas()
                for m in range(M):
                    for k in range(kn):
                        nc.tensor.matmul(
                            psums[m],
                            w1_t[:, k, m * P : (m + 1) * P],
                            xt_t[:, k, :],
                            start=(kdone + k == 0),
                            stop=(kdone + k == K - 1),
                        )
                kdone += kn
            for m in range(M):
                nc.scalar.activation(
                    h_sb[:, m, :], psums[m], AF.Relu, bias=b1_sb[:, m : m + 1]
                )
            # ---- layer 2: fps.T[H, BC] = w2.T @ h.T + b2 ----
            w2_t = w_pool.tile([P, 4, 512], FP16, tag="w1", name=f"w2_{name}")
            nc.sync.dma_start(
                w2_t[:, :M, :], w2[name].ap().rearrange("(ko p) m -> p ko m", p=P)
            )
            for m in range(4):
                ps = psum_mm.tile([P, BC], F32, tag="mmps", name=f"l2_{name}_{m}")
                for k in range(M):
                    nc.tensor.matmul(
                        ps,
                        w2_t[:, k, m * P : (m + 1) * P],
                        h_sb[:, k, :],
                        start=(k == 0),
                        stop=(k == M - 1),
                    )
                nc.scalar.activation(
                    fps16[:, ei * 4 + m, :], ps, AF.Identity, bias=b2_sb[:, m : m + 1]
                )
                # z-gate accumulation for the learned fusion weights
                nc.tensor.matmul(
                    z_ps,
                    wg_sb[:, ei * 4 + m, :],
                    fps16[:, ei * 4 + m, :],
                    start=(z_idx[0] == 0),
                    stop=(z_idx[0] == 19),
                )
                z_idx[0] += 1
            # ---- interleaved stats ----
            emit_ss(ei)
            if name != ORDER[-1]:
                for prev in done_encs:
                    emit_pair(prev, ei)
                    pair_count[0] += 1
                # partial mean-fallback sum (first 4 encoders), ping-pong
                if len(done_encs) == 1:
                    nc.gpsimd.tensor_add(
                        mpartA,
                        fps16[:, done_encs[0] * 4 : done_encs[0] * 4 + 4, :],
                        fps16[:, ei * 4 : ei * 4 + 4, :],
                    )
                elif len(done_encs) == 2:
                    nc.gpsimd.tensor_add(
                        meansum, mpartA, fps16[:, ei * 4 : ei * 4 + 4, :]
                    )
                elif len(done_encs) == 3:
                    nc.gpsimd.tensor_add(
                        mpartA, meansum, fps16[:, ei * 4 : ei * 4 + 4, :]
                    )
            done_encs.append(ei)

        # ---- everything below needs only ss / z / mc-fps: run it while ----
        # ---- the tail pair work (DVE/Pool) streams in parallel          ----
        last = LAST_EI
        # total mean-fallback sum
        nc.gpsimd.tensor_add(meansum, mpartA, fps16[:, last * 4 : last * 4 + 4, :])
        # tail pairs (prod on DVE feeds the d-matmuls quickly; s on Pool)
        for prev in done_encs[:-1]:
            emit_pair(prev, last)

        # ================= Phase B: pair softmax + fusion gate =============
        # l5 = ln(ss); pl = pcat @ l5; invnn = exp(-0.5*pl)   (no d needed)
        l5 = smalls.tile([5, BC], MID)
        nc.scalar.activation(l5, ss_sb, AF.Ln)
        pl_ps = psum_st.tile([10, BC], F32, tag="stps", name="pl")
        nc.tensor.matmul(pl_ps, pcat_sb, l5, start=True, stop=True)
        invnn = smalls.tile([10, BC], MID)
        nc.scalar.activation(invnn, pl_ps, AF.Exp, scale=-0.5)

        # fpw = softmax(z + wg_b) over the 5 encoders  (no d needed)
        ez = smalls.tile([5, BC], MID)
        nc.scalar.activation(ez, z_ps, AF.Exp, bias=wgb_sb)
        sez_ps = psum_st.tile([1, BC], F32, tag="stps", name="sez")
        nc.tensor.matmul(sez_ps, ones_col16[0:5, :], ez, start=True, stop=True)
        lnsez = smalls.tile([1, BC], MID)
        nc.scalar.activation(lnsez, sez_ps, AF.Ln)
        zc = smalls.tile([5, BC], MID)
        nc.scalar.activation(zc, z_ps, AF.Copy)
        fz_ps = psum_st.tile([5, BC], F32, tag="stps", name="fz")
        nc.tensor.matmul(fz_ps, eye10_sb[0:5, 0:5], zc, start=True, stop=False)
        nc.tensor.matmul(fz_ps, mones10_sb[:, 0:5], lnsez, start=False, stop=True)
        fpw_sb = smalls.tile([5, BC], MID)
        nc.scalar.activation(fpw_sb, fz_ps, AF.Exp, bias=wgb_sb)

        bc_idx = [0]

        def broadcast(dst, src_tile, row, nm):
            ksel = src_tile.shape[0]
            pool = [psum_bc, psum_st, psum_st][bc_idx[0] % 3]
            bc_idx[0] += 1
            bc_ps = pool.tile(
                [P, BC], F32, tag="bcps" if pool is psum_bc else "stps", name=nm
            )
            nc.tensor.matmul(
                bc_ps,
                esel_sb[0:ksel, row * P : (row + 1) * P],
                src_tile,
                start=True,
                stop=True,
            )
            nc.scalar.activation(dst, bc_ps, AF.Copy)

        for i in range(5):
            broadcast(fpwrep[:, i, :], fpw_sb, i, f"bc_fpw{i}")

        # wsum[ht] = sum_i fps[i,ht] * fpwrep[i] — only needs fpwrep, so it
        # runs here, fully overlapped with the wq chain below
        fps_by_ht = fps16.rearrange("p (i h) n -> p h i n", h=4)
        for ht in range(4):
            weng = nc.gpsimd if ht % 2 == 1 else nc.vector
            uw = late_pool.tile([P, 5, BC], MID, tag="uw5", name=f"uw_{ht}")
            weng.tensor_mul(uw, fps_by_ht[:, ht, :, :], fpwrep)
            u1 = late_pool.tile([P, 2, BC], MID, tag="u2", name=f"u1_{ht}")
            weng.tensor_add(u1, uw[:, 0:2, :], uw[:, 2:4, :])
            u2 = late_pool.tile([P, BC], MID, tag="u1w", name=f"u2_{ht}")
            weng.tensor_add(u2, u1[:, 0, :], u1[:, 1, :])
            weng.tensor_add(wsum[:, ht, :], u2, uw[:, 4, :])

        # wq chain (needs the tail-pair d stats)
        sims = smalls.tile([10, BC], MID)
        nc.vector.tensor_mul(sims, stats[0:10, :], invnn)
        mask10 = smalls.tile([10, BC], MID)
        nc.vector.tensor_scalar(
            mask10, in0=stats[0:10, :], scalar1=0.0, scalar2=None, op0=ALU.is_gt
        )
        e0 = smalls.tile([10, BC], MID)
        nc.scalar.activation(e0, sims, AF.Exp)
        e_sb = smalls.tile([10, BC], MID)
        nc.vector.tensor_mul(e_sb, mask10, e0)
        den_ps = psum_st.tile([1, BC], F32, tag="stps", name="den")
        nc.tensor.matmul(den_ps, ones_col16[0:10, :], e_sb, start=True, stop=True)
        # mean-fallback row: 0.2 iff no pair selected
        mfr = smalls.tile([1, BC], MID)
        nc.vector.tensor_scalar(
            mfr, in0=den_ps, scalar1=0.0, scalar2=0.2, op0=ALU.is_le, op1=ALU.mult
        )
        # wq = 0.5 * mask * exp(sims - ln(max(den,1)))
        den_c = smalls.tile([1, BC], F32)
        nc.vector.tensor_scalar_max(den_c, den_ps, 1.0)
        lnden = smalls.tile([1, BC], MID)
        nc.scalar.activation(lnden, den_c, AF.Ln)
        wqz_ps = psum_st.tile([10, BC], F32, tag="stps", name="wqz")
        nc.tensor.matmul(wqz_ps, eye10_sb, sims, start=True, stop=False)
        nc.tensor.matmul(wqz_ps, mones10_sb, lnden, start=False, stop=True)
        wq0 = smalls.tile([10, BC], MID)
        nc.scalar.activation(wq0, wqz_ps, AF.Exp, bias=ln05)
        wq_sb = smalls.tile([10, BC], MID)
        nc.vector.tensor_mul(wq_sb, mask10, wq0)

        broadcast(mfallrep, mfr, 0, "bc_mf")
        for ht in range(4):
            # mean-fallback term per ht, off the critical path (Pool)
            nc.gpsimd.tensor_mul(mpartA[:, ht, :], meansum[:, ht, :], mfallrep)
        for p in range(10):
            broadcast(wqrep[:, p, :], wq_sb, p, f"bc_wq{p}")

        # ================= Phase C: masked aggregation (wide fp16) =========
        ew_t = w_pool.tile([P, 4, 512], FP16, tag="w16", name="ew_t")
        nc.sync.dma_start(ew_t, enh_w.ap().rearrange("(ko p) m -> p ko m", p=P))
        fw_view = fus_w.ap().rearrange("(ko p) m -> p ko m", p=P)
        fw_lo = w_pool.tile([P, 4, 512], FP16, tag="w16", name="fw_lo")
        nc.sync.dma_start(fw_lo, fw_view[:, 0:4, :])
        fw_hi = w_pool.tile([P, 4, 512], FP16, tag="w16", name="fw_hi")
        nc.sync.dma_start(fw_hi, fw_view[:, 4:8, :])

        enh_ps = [
            psum_mm.tile([P, BC], F32, tag="mmps", name=f"enh_{m}") for m in range(4)
        ]
        for ht in range(4):
            # common[ht] = sum_p mcf[p,ht] * wqrep[p] + mfall * meansum[ht]
            mcfwq = w10_pool.tile([P, 10, BC], MID, tag="w10", name=f"mcfwq{ht}")
            nc.vector.tensor_mul(mcfwq, mcfw[:, ht, :, :], wqrep)
            t1 = late_pool.tile([P, 5, BC], MID, tag="t5", name=f"t1_{ht}")
            nc.vector.tensor_add(t1, mcfwq[:, 0:5, :], mcfwq[:, 5:10, :])
            t2 = late_pool.tile([P, 2, BC], MID, tag="t2", name=f"t2_{ht}")
            nc.vector.tensor_add(t2, t1[:, 0:2, :], t1[:, 2:4, :])
            t3 = late_pool.tile([P, BC], MID, tag="t3", name=f"t3_{ht}")
            nc.vector.tensor_add(t3, t2[:, 0, :], t2[:, 1, :])
            t4 = late_pool.tile([P, BC], MID, tag="t4", name=f"t4_{ht}")
            nc.vector.tensor_add(t4, t3, t1[:, 4, :])
            nc.vector.tensor_add(common[:, ht, :], t4, mpartA[:, ht, :])
            # enhance matmul accumulates as soon as common[ht] is ready
            for m in range(4):
                nc.tensor.matmul(
                    enh_ps[m],
                    ew_t[:, ht, m * P : (m + 1) * P],
                    common[:, ht, :],
                    start=(ht == 0),
                    stop=(ht == 3),
                )

        # ================= Phase D: enhance gate + fuse =================
        for m in range(4):
            gate = gate_pool.tile([P, BC], MID, tag="gate", name=f"gate{m}")
            nc.scalar.activation(gate, enh_ps[m], AF.Sigmoid, bias=enhb_sb[:, m : m + 1])
            nc.vector.tensor_mul(enh_sb[:, m, :], common[:, m, :], gate)

        out_view = out.ap().rearrange("(m p) n -> p m n", p=P)
        for m in range(4):
            ps = psum_mm.tile([P, BC], F32, tag="mmps", name=f"fus_{m}")
            for k in range(8):
                rhs = wsum[:, k, :] if k < 4 else enh_sb[:, k - 4, :]
                fw_t = fw_lo if k < 4 else fw_hi
                nc.tensor.matmul(
                    ps,
                    fw_t[:, k % 4, m * P : (m + 1) * P],
                    rhs,
                    start=(k == 0),
                    stop=(k == 7),
                )
            o_sb = gate_pool.tile([P, BC], F32, tag="osb", name=f"osb{m}")
            nc.scalar.activation(o_sb, ps, AF.Identity, bias=fusb_sb[:, m : m + 1])
            nc.sync.dma_start(out_view[:, m, :], o_sb)


def prep_inputs(inputs):
    """Host-side: build the per-core in_maps from full inputs."""
    x = np.asarray(inputs["fp_features"], np.float32)

    def pad_rows(a, rows):
        a = np.asarray(a, np.float32)
        if a.shape[0] == rows:
            return a.astype(np.float16)
        out = np.zeros((rows, a.shape[1]), np.float16)
        out[: a.shape[0]] = a.astype(np.float16)
        return out

    # padded transposed x (fp16), shared prep then per-core column slices
    xt_full = np.zeros((XT_K * P, B), np.float16)
    offs_in = np.cumsum([0, AP_D, MA_D, MB_D, MC_D])
    for ei, (name, din, K, dh) in enumerate(ENCS):
        seg = x[:, offs_in[ei] : offs_in[ei] + din]  # [B, din]
        xt_full[XT_OFF[ei] * P : XT_OFF[ei] * P + din, :] = np.ascontiguousarray(
            seg.T
        ).astype(np.float16)

    common_map = {}
    for ei, (name, din, K, dh) in enumerate(ENCS):
        common_map[f"w1_{name}"] = pad_rows(inputs[f"{name}_w1"], K * P)
        common_map[f"w2_{name}"] = np.asarray(inputs[f"{name}_w2"], np.float16)
        common_map[f"b1_{name}"] = (
            np.asarray(inputs[f"{name}_b1"], np.float32).reshape(dh // P, P).T.copy()
        )
        common_map[f"b2_{name}"] = (
            np.asarray(inputs[f"{name}_b2"], np.float32).reshape(4, P).T.copy()
        )
    wgw = np.asarray(inputs["wg_w"], np.float32).astype(np.float16)  # [2560, 5]
    common_map["wg_w"] = np.ascontiguousarray(
        wgw.reshape(20, 128, 5).transpose(1, 0, 2).reshape(128, 100))
    common_map["wg_b"] = np.asarray(inputs["wg_b"], np.float32).reshape(5, 1)
    pcat = np.zeros((5, 10), np.float16)
    for p in range(10):
        pcat[_I[p], p] = 1.0
        pcat[_J[p], p] = 1.0
    common_map["pcat"] = pcat
    esel = np.zeros((10, 10 * 128), np.float16)
    for p in range(10):
        esel[p, p * 128 : (p + 1) * 128] = 1.0
    common_map["esel"] = esel
    common_map["eye10"] = np.eye(10, dtype=np.float16)
    common_map["mones10"] = np.full((1, 10), -1.0, np.float16)
    common_map["enh_w"] = np.asarray(inputs["enh_w"], np.float16)
    common_map["enh_b"] = np.asarray(inputs["enh_b"], np.float32).reshape(4, P).T.copy()
    common_map["fus_w"] = np.asarray(inputs["fus_w"], np.float16)
    common_map["fus_b"] = np.asarray(inputs["fus_b"], np.float32).reshape(4, P).T.copy()

    in_maps = []
    for c in range(N_CORES):
        m = dict(common_map)
        m["xt"] = np.ascontiguousarray(xt_full[:, c * BC : (c + 1) * BC])
        in_maps.append(m)
    return in_maps


_NC_CACHE = None


def kernel(**inputs) -> np.ndarray:
    global _NC_CACHE
    if _NC_CACHE is None:
        _NC_CACHE = build_bass()
    nc = _NC_CACHE
    in_maps = prep_inputs(inputs)
    res = run_bass_kernel_spmd(nc, in_maps, core_ids=list(range(N_CORES)))
    outs = [res.results[c]["out"] for c in range(N_CORES)]  # each [H, BC]
    full = np.concatenate([o.T for o in outs], axis=0)  # [B, H]
    return np.ascontiguousarray(full.astype(np.float32))
